# revision 1
# baseline (speedup 1.0000x reference)
"""BuildingBlockEmbedder GNN kernel for trn2 — shared library.

Layout: feature-on-partition ("transposed") everywhere on device.
Per core: 64 building blocks x 32 atoms = 2048 nodes, 40960 edges.
"""
import numpy as np
import ml_dtypes

BF16 = ml_dtypes.bfloat16

# problem constants
NUM_GAUSS = 64
MAX_R = 5.0
L = 4
C = 256            # node/hidden channels
NPB = 32           # atoms per block
K = 20             # neighbors
B = 512            # blocks
N = B * NPB
E = N * K
GAUSS_COEFF = -0.5 / (MAX_R / (NUM_GAUSS - 1)) ** 2
NCORES = 8
BPC = B // NCORES          # 64 blocks per core
NPC = BPC * NPB            # 2048 nodes per core
EPB = NPB * K              # 640 edges per block
HALF = EPB // 2            # 320-edge matmul unit

# ---------------------------------------------------------------- host prep

def host_prep(local_coords, atom_types, edge_index, batch_bb, atom_embed, offset,
              We1, be1, We2, be2, Wn1, bn1, Wn2, bn2):
    """Build per-core device inputs from full problem inputs (all numpy)."""
    pos = np.asarray(local_coords, np.float32)
    types = np.asarray(atom_types).astype(np.int64)
    ei = np.asarray(edge_index).astype(np.int64)
    We1 = np.asarray(We1, np.float32); be1 = np.asarray(be1, np.float32)
    We2 = np.asarray(We2, np.float32); be2 = np.asarray(be2, np.float32)
    Wn1 = np.asarray(Wn1, np.float32); bn1 = np.asarray(bn1, np.float32)
    Wn2 = np.asarray(Wn2, np.float32); bn2 = np.asarray(bn2, np.float32)
    emb = np.asarray(atom_embed, np.float32)

    row, col = ei[0], ei[1]
    # structural assumptions from the reference graph builder
    assert np.array_equal(row, np.repeat(np.arange(N, dtype=np.int64), K)), \
        "edge rows must be repeat(arange(N), K)"
    assert np.all(col // NPB == row // NPB), "edges must stay within blocks"

    dvec = pos[col] - pos[row]
    d = np.sqrt((dvec * dvec).sum(-1))          # [E] Angstrom
    assert d.max() < MAX_R - 0.55, f"d.max()={d.max()}: last gaussian not negligible"
    radial = (0.01 * d * d).astype(np.float32)  # ANG_TO_NM^2 * d^2
    # gaussians 0..62 (63rd is exp(<-30) ~= 0 for all d here; its row carries radial)
    off = np.asarray(offset, np.float32)
    gauss = np.exp(GAUSS_COEFF * (d[:, None] - off[None, :63]) ** 2).astype(np.float32)

    # CRG [B, 128, EPB]: rows 0-31 C_sel, 32-63 R_sel, 64 radial, 65-127 gauss
    col_local = (col - (row // NPB) * NPB).astype(np.int32).reshape(B, EPB)
    crg = np.zeros((B, 128, EPB), np.float32)
    e_ar = np.arange(EPB)
    r_sel = np.zeros((NPB, EPB), np.float32)
    r_sel[e_ar // K, e_ar] = 1.0
    for b in range(B):
        crg[b, col_local[b], e_ar] = 1.0      # C_sel
    crg[:, 32:64, :] = r_sel[None]
    crg[:, 64, :] = radial.reshape(B, EPB)
    crg[:, 65:, :] = gauss.reshape(B, EPB, 63).transpose(0, 2, 1)
    crg = crg.astype(BF16)

    h0 = emb[types - 1]                        # [N, C] f32
    h0T = h0.reshape(NCORES, NPC, C).transpose(0, 2, 1).reshape(
        NCORES, 2, 128, NPC).copy()            # [core, chunk, 128, 2048]

    def chunks_lhsT(w):   # w [L, 256, 256] -> [L, kc, mc, 128, 128] bf16
        return np.ascontiguousarray(
            w.reshape(L, 2, 128, 2, 128).transpose(0, 1, 3, 2, 4)).astype(BF16)

    w1b = np.ascontiguousarray(
        We1[:, 256:512, :].reshape(L, 2, 128, 256))      # rhs [L, kc, 128, 256]
    w1a = np.ascontiguousarray(We1[:, 0:256, :].reshape(L, 2, 128, 256))
    w1ab = np.stack([w1b, w1a], axis=1).astype(BF16)     # [L, 2(b,a), kc, 128, 256]
    w1dp = np.concatenate([We1[:, 512:513, :], We1[:, 513:576, :]],
                          axis=1).astype(BF16)           # [L, 64, 256]
    we2 = chunks_lhsT(We2)
    wn1a = chunks_lhsT(Wn1[:, 0:256, :])
    wn1b = chunks_lhsT(Wn1[:, 256:512, :])
    wn2 = chunks_lhsT(Wn2)
    # bias K=1 lhsT rows [L, 2(bn1,bn2), mc, 1, 128] bf16
    bnrow = np.stack([bn1, bn2], axis=1).reshape(L, 2, 2, 1, 128).astype(BF16)
    # ACT/DVE bias columns [128, 4*L*2]; col = (j*L + l)*2 + mc
    # j: 0 = be1, 1 = be2, 2 = -be2, 3 = -be1
    be12 = np.zeros((128, 4 * L * 2), np.float32)
    for j, bb in enumerate([be1, be2, -be2, -be1]):
        for l in range(L):
            for mc in range(2):
                be12[:, (j * L + l) * 2 + mc] = bb[l, mc * 128:(mc + 1) * 128]

    def pmaj(w):  # [L, kc, mc, 128p, 128q] -> [128, L*kc*mc*128]
        return np.ascontiguousarray(w.transpose(3, 0, 1, 2, 4).reshape(128, -1))

    # v2 extras: We2 as rhs [128, L*2*256]; be2 repeated row; Ssel matrices
    we2r = np.ascontiguousarray(
        We2.reshape(L, 2, 128, 256).transpose(2, 0, 1, 3).reshape(128, -1)).astype(BF16)
    be2row = np.ascontiguousarray(
        np.repeat(be2[:, None, :], 2, axis=1).reshape(1, L * 512)).astype(BF16)
    ssel = np.zeros((128, 5 * NPB), np.float32)
    for ec in range(5):
        j = np.arange(128)
        ssel[j, ec * NPB + (ec * 128 + j) // K] = 1.0
    ssel = np.ascontiguousarray(ssel).astype(BF16)

    shared = dict(
        we2r=we2r, be2row=be2row, ssel=ssel,
        be2_nonzero=np.asarray([1.0 if np.any(be2 != 0) else 0.0], np.float32),
        we2=pmaj(we2),
        w1ab=np.ascontiguousarray(w1ab.transpose(3, 0, 1, 2, 4).reshape(128, -1)),
        w1dp=np.ascontiguousarray(w1dp.transpose(1, 0, 2).reshape(64, -1)),
        wn1a=pmaj(wn1a), wn1b=pmaj(wn1b), wn2=pmaj(wn2),
        bnrow=np.ascontiguousarray(bnrow.transpose(3, 0, 1, 2, 4).reshape(1, -1)),
        be12=be12)
    per_core = []
    for c in range(NCORES):
        m = dict(h0T=h0T[c], crg=crg[c * BPC:(c + 1) * BPC])
        m.update(shared)
        per_core.append(m)
    return per_core


def host_unshard(results):
    """results: list of 8 dicts with 'poolT' [2,128,nb] -> full [B, 256] f32."""
    outs = []
    for r in results:
        pt = np.asarray(r["poolT"], np.float32)      # [2, 128, nb]
        nb = pt.shape[2]
        outs.append(pt.reshape(256, nb).T)           # [nb, 256]
    return np.concatenate(outs, axis=0)


# ------------------------------------------------------------ tile drain fix

def apply_tilefix():
    """This container's walrus allows only ONE sem-wait on an SP Drain —
    split the Tile tail-drain waits across serial drains."""
    import concourse.mybir as mybir
    import concourse.tile as tile
    from concourse.tile import ScopedClock

    if getattr(tile.TileContext, "_drain_fix_applied", False):
        return

    def _split(self, tick_clock, wait_clock):
        d = self.nc.sync.drain()
        wait_clock.add_sem_waits(d.ins, ScopedClock({None: tick_clock.global_clock}))
        ws = list(d.ins.sync_info.on_wait) if d.ins.sync_info is not None else []
        if len(ws) > 1:
            d.ins.sync_info.on_wait = ws[:1]
            for w in ws[1:]:
                e = self.nc.sync.drain()
                e.ins.sync_info = mybir.SyncInfo(on_update=[], on_wait=[w])
        self.nc.all_engine_barrier()
        assert self.sems is not None
        popped = self.nc._tile_sem_poison_stack.pop()
        assert popped is self._sem_poison
        self.nc.clear_and_free_semaphores(list(self.sems.allocated().values()))
        self.nc.all_engine_barrier()

    tile.TileContext._drain_and_barrier = _split
    tile.TileContext._drain_fix_applied = True


# ---------------------------------------------------- wait-splitting post-pass

def split_waits(nc, cap=1, cap_sp=1):
    """walrus in this container caps sem-waits per instruction. Hoist excess
    waits onto same-engine NOPs emitted just before the instruction."""
    import concourse.mybir as mybir
    k = 0
    for fn in nc.m.functions:
        for bb in fn.blocks:
            out = []
            for inst in bb.instructions:
                si = inst.sync_info
                ws = list(si.on_wait) if si is not None else []
                c = cap_sp if inst.engine == mybir.EngineType.SP else cap
                if len(ws) > c:
                    keep = ws[:c] if c > 0 else []
                    rest = ws[c:] if c > 0 else ws
                    while rest:
                        chunk, rest = rest[:max(c, 1)], rest[max(c, 1):]
                        nop = mybir.InstNoOp(
                            name=f"wsplit-{k}", engine=inst.engine,
                            sync_info=mybir.SyncInfo(on_wait=chunk, on_update=[]),
                            bass_nofuse=True)
                        k += 1
                        out.append(nop)
                    inst.sync_info.on_wait = keep
                out.append(inst)
            bb.instructions[:] = out
    return k


# ------------------------------------------------------------- bass builder

def build_nc(nb=BPC, reps=1, hw_loop=False):
    """Build the per-core Bass module. nb = blocks per core (small for sim)."""
    import concourse.bass as bass
    import concourse.mybir as mybir
    import concourse.tile as tile

    f32, bf16 = mybir.dt.float32, mybir.dt.bfloat16
    AF = mybir.ActivationFunctionType
    ALU = mybir.AluOpType
    nn = nb * NPB                     # nodes this build
    nts = min(512, nn)                # node tile size
    nt = nn // nts                    # node tiles

    nc = bass.Bass()
    h0T_d = nc.dram_tensor("h0T", [2, 128, nn], f32, kind="ExternalInput")
    crg_d = nc.dram_tensor("crg", [nb, 128, EPB], bf16, kind="ExternalInput")
    we2_d = nc.dram_tensor("we2", [128, L * 2 * 2 * 128], bf16, kind="ExternalInput")
    w1ab_d = nc.dram_tensor("w1ab", [128, L * 2 * 2 * 256], bf16, kind="ExternalInput")
    w1dp_d = nc.dram_tensor("w1dp", [64, L * 256], bf16, kind="ExternalInput")
    wn1a_d = nc.dram_tensor("wn1a", [128, L * 2 * 2 * 128], bf16, kind="ExternalInput")
    wn1b_d = nc.dram_tensor("wn1b", [128, L * 2 * 2 * 128], bf16, kind="ExternalInput")
    wn2_d = nc.dram_tensor("wn2", [128, L * 2 * 2 * 128], bf16, kind="ExternalInput")
    bnrow_d = nc.dram_tensor("bnrow", [1, L * 2 * 2 * 128], bf16, kind="ExternalInput")
    be12_d = nc.dram_tensor("be12", [128, 4 * L * 2], f32, kind="ExternalInput")
    out_d = nc.dram_tensor("poolT", [2, 128, nb], f32, kind="ExternalOutput")

    with tile.TileContext(nc) as tc:
        with (
            tc.tile_pool(name="const", bufs=1) as csp,
            tc.tile_pool(name="crgp", bufs=1) as crgp,
            tc.tile_pool(name="state", bufs=1) as stp,
            tc.tile_pool(name="comb", bufs=4) as combp,
            tc.tile_pool(name="m1p", bufs=3) as m1p,
            tc.tile_pool(name="m2p", bufs=6) as m2p,
            tc.tile_pool(name="ps", bufs=8, space="PSUM") as psp,
        ):
            # ---- resident constants
            we2_s = csp.tile([128, L * 2 * 2 * 128], bf16, name="we2_s")
            nc.sync.dma_start(we2_s[:], we2_d[:])
            w1ab_s = csp.tile([128, L * 2 * 2 * 256], bf16, name="w1ab_s")
            nc.sync.dma_start(w1ab_s[:], w1ab_d[:])
            w1dp_s = csp.tile([128, L * 256], bf16, name="w1dp_s")
            nc.sync.dma_start(w1dp_s[64:128, :], w1dp_d[:])
            wn1a_s = csp.tile([128, L * 2 * 2 * 128], bf16, name="wn1a_s")
            nc.sync.dma_start(wn1a_s[:], wn1a_d[:])
            wn1b_s = csp.tile([128, L * 2 * 2 * 128], bf16, name="wn1b_s")
            nc.sync.dma_start(wn1b_s[:], wn1b_d[:])
            wn2_s = csp.tile([128, L * 2 * 2 * 128], bf16, name="wn2_s")
            nc.sync.dma_start(wn2_s[:], wn2_d[:])
            bnrow_s = csp.tile([128, L * 2 * 2 * 128], bf16, name="bnrow_s")
            nc.sync.dma_start(bnrow_s[0:1, :], bnrow_d[:])
            be12_s = csp.tile([128, 4 * L * 2], f32, name="be12_s")
            nc.sync.dma_start(be12_s[:], be12_d[:])
            ones_s = csp.tile([128, 512], bf16, name="ones_s")
            nc.gpsimd.memset(ones_s[0:1, :], 1.0)

            def we2_ap(l, kc, mc):
                o = ((l * 2 + kc) * 2 + mc) * 128
                return we2_s[:, o:o + 128]

            def w1ab_ap(l, s, kc):
                o = ((l * 2 + s) * 2 + kc) * 256
                return w1ab_s[:, o:o + 256]

            def wfam_ap(t, l, kc, mc):
                o = ((l * 2 + kc) * 2 + mc) * 128
                return t[:, o:o + 128]

            def bnrow_ap(l, j, mc):
                o = ((l * 2 + j) * 2 + mc) * 128
                return bnrow_s[0:1, o:o + 128]

            def be_ap(j, l, mc):
                o = (j * L + l) * 2 + mc
                return be12_s[:, o:o + 1]

            # ---- CRG resident
            crg_s = []
            for b in range(nb):
                t = crgp.tile([128, EPB], bf16, name=f"crg{b}", tag=f"crg{b}")
                nc.sync.dma_start(t[:], crg_d[b])
                crg_s.append(t)

            # ---- state
            hT, hbf, aggT, aggbf, n1bf = [], [], [], [], []
            for c in range(2):
                t = stp.tile([128, nn], f32, name=f"hT{c}", tag=f"hT{c}")
                hT.append(t)
                hbf.append(stp.tile([128, nn], bf16, name=f"hbf{c}", tag=f"hbf{c}"))
                aggT.append(stp.tile([128, nn], f32, name=f"aggT{c}", tag=f"aggT{c}"))
                aggbf.append(stp.tile([128, nn], bf16, name=f"aggbf{c}", tag=f"agb{c}"))
                n1bf.append(stp.tile([128, nn], bf16, name=f"n1bf{c}", tag=f"n1b{c}"))

            import contextlib
            loop_ctx = (tc.For_i(0, reps, 1) if hw_loop
                        else contextlib.nullcontext())
            rep_range = range(1 if hw_loop else reps)
            with loop_ctx:
             for rep in rep_range:
              for c in range(2):
                nc.sync.dma_start(hT[c][:], h0T_d[c])
              for l in range(L):
                for c in range(2):
                    nc.gpsimd.tensor_copy(hbf[c][:], hT[c][:])
                # ---------------- edge phase, per block
                for b in range(nb):
                    ps_ab = psp.tile([128, 512], f32, tag="ps", name=f"ab{l}_{b}")
                    for sel, pos0 in ((0, 0), (1, 32)):
                        for kc in range(2):
                            nc.tensor.matmul(ps_ab[pos0:pos0 + 32, 0:256],
                                             lhsT=hbf[kc][:, b * NPB:(b + 1) * NPB],
                                             rhs=w1ab_ap(l, sel, kc),
                                             start=(kc == 0), stop=(kc == 1),
                                             tile_position=(0, pos0))
                    comb = combp.tile([128, 256], bf16, tag="comb", name=f"cb{l}_{b}")
                    nc.scalar.copy(comb[0:64, :], ps_ab[0:64, 0:256])
                    nc.gpsimd.tensor_copy(comb[64:128, :],
                                          w1dp_s[64:128, l * 256:(l + 1) * 256])
                    m1t = [m1p.tile([128, EPB], bf16, tag=f"m1_{kc}",
                                    name=f"m1_{l}_{b}_{kc}") for kc in range(2)]
                    for mc in range(2):
                        for h in range(2):
                            ps1 = psp.tile([128, 512], f32, tag="ps",
                                           name=f"p1_{l}_{b}_{mc}_{h}")
                            nc.tensor.matmul(ps1[:, 0:HALF],
                                             lhsT=comb[:, mc * 128:(mc + 1) * 128],
                                             rhs=crg_s[b][:, h * HALF:(h + 1) * HALF],
                                             start=True, stop=True)
                            nc.scalar.activation(m1t[mc][:, h * HALF:(h + 1) * HALF],
                                                 ps1[:, 0:HALF], AF.Relu,
                                                 bias=be_ap(0, l, mc))
                    for mc in range(2):
                        for h in range(2):
                            ps2 = psp.tile([128, 512], f32, tag="ps",
                                           name=f"p2_{l}_{b}_{mc}_{h}")
                            for kc in range(2):
                                nc.tensor.matmul(
                                    ps2[:, 0:HALF],
                                    lhsT=we2_ap(l, kc, mc),
                                    rhs=m1t[kc][:, h * HALF:(h + 1) * HALF],
                                    start=(kc == 0), stop=(kc == 1))
                            m2t = m2p.tile([128, HALF], bf16, tag="m2",
                                           name=f"m2_{l}_{b}_{mc}_{h}")
                            if (b * 4 + mc * 2 + h) % 2:
                                nc.scalar.activation(m2t[:], ps2[:, 0:HALF], AF.Relu,
                                                     bias=be_ap(1, l, mc))
                            else:
                                # relu(x + b) == max(x, -b) + b
                                nc.vector.scalar_tensor_tensor(
                                    m2t[:], ps2[:, 0:HALF], be_ap(2, l, mc),
                                    be_ap(1, l, mc).to_broadcast([128, HALF]),
                                    op0=ALU.max, op1=ALU.add)
                            nc.vector.tensor_reduce(
                                aggT[mc][:, b * NPB + h * 16: b * NPB + (h + 1) * 16],
                                m2t[:].rearrange("p (n k) -> p n k", k=K),
                                axis=mybir.AxisListType.X, op=ALU.add)
                # ---------------- node phase
                for c in range(2):
                    nc.gpsimd.tensor_copy(aggbf[c][:], aggT[c][:])
                for mc in range(2):
                    for t in range(nt):
                        sl = slice(t * nts, (t + 1) * nts)
                        psn = psp.tile([128, 512], f32, tag="ps",
                                       name=f"n1_{l}_{mc}_{t}")
                        nc.tensor.matmul(psn[:, 0:nts], lhsT=bnrow_ap(l, 0, mc),
                                         rhs=ones_s[0:1, 0:nts], start=True, stop=False)
                        for kc in range(2):
                            nc.tensor.matmul(psn[:, 0:nts],
                                             lhsT=wfam_ap(wn1a_s, l, kc, mc),
                                             rhs=hbf[kc][:, sl], start=False, stop=False)
                            nc.tensor.matmul(psn[:, 0:nts],
                                             lhsT=wfam_ap(wn1b_s, l, kc, mc),
                                             rhs=aggbf[kc][:, sl], start=False,
                                             stop=(kc == 1))
                        nc.scalar.activation(n1bf[mc][:, sl], psn[:, 0:nts], AF.Relu)
                for mc in range(2):
                    for t in range(nt):
                        sl = slice(t * nts, (t + 1) * nts)
                        pso = psp.tile([128, 512], f32, tag="ps",
                                       name=f"n2_{l}_{mc}_{t}")
                        nc.tensor.matmul(pso[:, 0:nts], lhsT=bnrow_ap(l, 1, mc),
                                         rhs=ones_s[0:1, 0:nts], start=True, stop=False)
                        for kc in range(2):
                            nc.tensor.matmul(pso[:, 0:nts],
                                             lhsT=wfam_ap(wn2_s, l, kc, mc),
                                             rhs=n1bf[kc][:, sl], start=False,
                                             stop=(kc == 1))
                        nc.vector.scalar_tensor_tensor(
                            hT[mc][:, sl], hT[mc][:, sl], 2.0, pso[:, 0:nts],
                            op0=ALU.mult, op1=ALU.add)
              # ---------------- pooling
              for mc in range(2):
                pool_t = stp.tile([128, nb], f32, tag=f"pool{mc}", name=f"pool{mc}")
                nc.vector.tensor_reduce(pool_t[:],
                                        hT[mc][:].rearrange("p (n k) -> p n k", k=NPB),
                                        axis=mybir.AxisListType.X, op=ALU.add)
                nc.scalar.mul(pool_t[:], pool_t[:], 1.0 / NPB)
                nc.sync.dma_start(out_d[mc], pool_t[:])
    return nc


# --------------------------------------------------- numpy model of the math

def numpy_model(ins, nb=BPC, cores=None):
    """Replicate the device math (incl. bf16 rounding) for validation.
    ins: list of per-core input dicts (from host_prep). Returns [sum_nb*NCORES? , 256]."""
    outs = []
    for m in (ins if cores is None else [ins[c] for c in cores]):
        h = np.asarray(m["h0T"], np.float32).reshape(256, -1)[:, :nb * NPB]  # [256, nn]
        crg = np.asarray(m["crg"], np.float32)[:nb]
        L4 = L
        we2 = np.asarray(m["we2"], np.float32).reshape(128, L4, 2, 2, 128).transpose(1, 2, 3, 0, 4)
        w1ab = np.asarray(m["w1ab"], np.float32).reshape(128, L4, 2, 2, 256).transpose(1, 2, 3, 0, 4)
        w1dp = np.asarray(m["w1dp"], np.float32).reshape(64, L4, 256).transpose(1, 0, 2)
        wn1a = np.asarray(m["wn1a"], np.float32).reshape(128, L4, 2, 2, 128).transpose(1, 2, 3, 0, 4)
        wn1b = np.asarray(m["wn1b"], np.float32).reshape(128, L4, 2, 2, 128).transpose(1, 2, 3, 0, 4)
        wn2 = np.asarray(m["wn2"], np.float32).reshape(128, L4, 2, 2, 128).transpose(1, 2, 3, 0, 4)
        bnrow = np.asarray(m["bnrow"], np.float32).reshape(1, L4, 2, 2, 128).transpose(1, 2, 3, 0, 4)
        be12 = np.asarray(m["be12"], np.float32)
        nn = nb * NPB

        def b16(x):
            return x.astype(BF16).astype(np.float32)

        def blk(w):  # [kc, mc, 128, 128] -> [256, 256]
            return np.concatenate(
                [np.concatenate([w[kc_, mc_] for mc_ in range(2)], axis=1)
                 for kc_ in range(2)], axis=0)

        for l in range(L):
            hb = b16(h)                                    # [256, nn]
            # hAB per block
            W1b = np.concatenate([w1ab[l, 0, kc_] for kc_ in range(2)], axis=0)
            W1a = np.concatenate([w1ab[l, 1, kc_] for kc_ in range(2)], axis=0)
            be1 = np.concatenate([be12[:, (0 * L + l) * 2 + mc_] for mc_ in range(2)])
            be2 = np.concatenate([be12[:, (1 * L + l) * 2 + mc_] for mc_ in range(2)])
            agg = np.zeros((256, nn), np.float32)
            for b in range(nb):
                hs = hb[:, b * NPB:(b + 1) * NPB]          # [256, 32]
                hB = b16(hs.T @ W1b)                       # [32, 256] evicted bf16
                hA = b16(hs.T @ W1a)
                combined = np.concatenate([hB, hA, w1dp[l]], axis=0)  # [128, 256]
                pre1 = combined.T @ crg[b]                 # [256, EPB]
                m1 = b16(np.maximum(pre1 + be1[:, None], 0.0))
                W2 = blk(we2[l])
                m2 = b16(np.maximum(W2.T @ m1 + be2[:, None], 0.0))
                agg[:, b * NPB:(b + 1) * NPB] = (
                    m2.reshape(256, NPB, K).sum(axis=2))
            aggb = b16(agg)
            N1a, N1b_, N2 = blk(wn1a[l]), blk(wn1b[l]), blk(wn2[l])
            bn1 = bnrow[l, 0].reshape(256)
            bn2 = bnrow[l, 1].reshape(256)
            n1 = b16(np.maximum(N1a.T @ hb + N1b_.T @ aggb + bn1[:, None], 0.0))
            out = N2.T @ n1 + bn2[:, None]
            h = 2.0 * h + out
        pooled = h.reshape(256, nb, NPB).mean(axis=2)       # [256, nb]
        outs.append(pooled.T)
    return np.concatenate(outs, axis=0)


# --------------------------------------------------------------- builder v2
# m2 in normal layout (edges on partitions); segment-sum as PE matmuls with
# constant Ssel matrices; agg evicted straight to bf16.

def build_nc_v2(nb=BPC, reps=1, hw_loop=False, be2_mm=False,
                m1_dve_of_8=2, m2_dve_of_8=5, comb_dve_of_8=0, agg_dve_of_8=0):
    import contextlib
    import concourse.bass as bass
    import concourse.mybir as mybir
    import concourse.tile as tile

    f32, bf16 = mybir.dt.float32, mybir.dt.bfloat16
    AF = mybir.ActivationFunctionType
    ALU = mybir.AluOpType
    nn = nb * NPB
    nts = min(512, nn)
    nt = nn // nts

    nc = bass.Bass()
    h0T_d = nc.dram_tensor("h0T", [2, 128, nn], f32, kind="ExternalInput")
    crg_d = nc.dram_tensor("crg", [nb, 128, EPB], bf16, kind="ExternalInput")
    we2r_d = nc.dram_tensor("we2r", [128, L * 2 * 256], bf16, kind="ExternalInput")
    w1ab_d = nc.dram_tensor("w1ab", [128, L * 2 * 2 * 256], bf16, kind="ExternalInput")
    w1dp_d = nc.dram_tensor("w1dp", [64, L * 256], bf16, kind="ExternalInput")
    wn1a_d = nc.dram_tensor("wn1a", [128, L * 2 * 2 * 128], bf16, kind="ExternalInput")
    wn1b_d = nc.dram_tensor("wn1b", [128, L * 2 * 2 * 128], bf16, kind="ExternalInput")
    wn2_d = nc.dram_tensor("wn2", [128, L * 2 * 2 * 128], bf16, kind="ExternalInput")
    bnrow_d = nc.dram_tensor("bnrow", [1, L * 2 * 2 * 128], bf16, kind="ExternalInput")
    be12_d = nc.dram_tensor("be12", [128, 4 * L * 2], f32, kind="ExternalInput")
    be2row_d = nc.dram_tensor("be2row", [1, L * 512], bf16, kind="ExternalInput")
    ssel_d = nc.dram_tensor("ssel", [128, 5 * NPB], bf16, kind="ExternalInput")
    out_d = nc.dram_tensor("poolT", [2, 128, nb], f32, kind="ExternalOutput")

    with tile.TileContext(nc) as tc:
        with (
            tc.tile_pool(name="const", bufs=1) as csp,
            tc.tile_pool(name="crgp", bufs=1) as crgp,
            tc.tile_pool(name="state", bufs=1) as stp,
            tc.tile_pool(name="comb", bufs=1) as combp,
            tc.tile_pool(name="m1p", bufs=5) as m1p,
            tc.tile_pool(name="m2p", bufs=14) as m2p,
            tc.tile_pool(name="ps", bufs=7, space="PSUM") as psp,
            tc.tile_pool(name="psagg", bufs=1, space="PSUM") as psaggp,
        ):
            we2r_s = csp.tile([128, L * 2 * 256], bf16, name="we2r_s")
            nc.sync.dma_start(we2r_s[:], we2r_d[:])
            w1ab_s = csp.tile([128, L * 2 * 2 * 256], bf16, name="w1ab_s")
            nc.sync.dma_start(w1ab_s[:], w1ab_d[:])
            w1dp_s = csp.tile([128, L * 256], bf16, name="w1dp_s")
            nc.sync.dma_start(w1dp_s[64:128, :], w1dp_d[:])
            wn1a_s = csp.tile([128, L * 2 * 2 * 128], bf16, name="wn1a_s")
            nc.sync.dma_start(wn1a_s[:], wn1a_d[:])
            wn1b_s = csp.tile([128, L * 2 * 2 * 128], bf16, name="wn1b_s")
            nc.sync.dma_start(wn1b_s[:], wn1b_d[:])
            wn2_s = csp.tile([128, L * 2 * 2 * 128], bf16, name="wn2_s")
            nc.sync.dma_start(wn2_s[:], wn2_d[:])
            bnrow_s = csp.tile([128, L * 2 * 2 * 128], bf16, name="bnrow_s")
            nc.sync.dma_start(bnrow_s[0:1, :], bnrow_d[:])
            be12_s = csp.tile([128, 4 * L * 2], f32, name="be12_s")
            nc.sync.dma_start(be12_s[:], be12_d[:])
            be2row_s = csp.tile([128, L * 512], bf16, name="be2row_s")
            nc.sync.dma_start(be2row_s[0:1, :], be2row_d[:])
            ssel_s = csp.tile([128, 5 * NPB], bf16, name="ssel_s")
            nc.sync.dma_start(ssel_s[:], ssel_d[:])
            ones_s = csp.tile([128, 512], bf16, name="ones_s")
            nc.gpsimd.memset(ones_s[0:1, :], 1.0)
            zcol_s = csp.tile([128, 1], f32, name="zcol_s")
            nc.gpsimd.memset(zcol_s[:], 0.0)

            def we2r_ap(l, kc):
                o = (l * 2 + kc) * 256
                return we2r_s[:, o:o + 256]

            def w1ab_ap(l, sel, kc):
                o = ((l * 2 + sel) * 2 + kc) * 256
                return w1ab_s[:, o:o + 256]

            def wfam_ap(t, l, kc, mc):
                o = ((l * 2 + kc) * 2 + mc) * 128
                return t[:, o:o + 128]

            def bnrow_ap(l, j, mc):
                o = ((l * 2 + j) * 2 + mc) * 128
                return bnrow_s[0:1, o:o + 128]

            def be_ap(j, l, mc):
                o = (j * L + l) * 2 + mc
                return be12_s[:, o:o + 1]

            hT, hbf, aggbf, n1bf = [], [], [], []
            for c in range(2):
                hT.append(stp.tile([128, nn], f32, name=f"hT{c}", tag=f"hT{c}"))
                hbf.append(stp.tile([128, nn], bf16, name=f"hbf{c}", tag=f"hbf{c}"))
                aggbf.append(stp.tile([128, nn], bf16, name=f"agb{c}", tag=f"agb{c}"))
                n1bf.append(stp.tile([128, nn], bf16, name=f"n1b{c}", tag=f"n1b{c}"))

            if not hw_loop:
                for c in range(2):
                    for t in range(nt):
                        sl = slice(t * nts, (t + 1) * nts)
                        nc.sync.dma_start(hT[c][:, sl], h0T_d[c][:, sl])
                        nc.gpsimd.tensor_copy(hbf[c][:, sl], hT[c][:, sl])

            crg_s = []
            for b in range(nb):
                t = crgp.tile([128, EPB], bf16, name=f"crg{b}", tag=f"crg{b}")
                nc.sync.dma_start(t[:], crg_d[b])
                crg_s.append(t)


            comb_tiles = [
                [combp.tile([128, 256], bf16, tag=f"comb{l}_{i}",
                            name=f"comb{l}_{i}") for i in range(min(4, nb))]
                for l in range(L)]

            evict_i = [0]

            def evict(out_ap, ps_ap, relu, bias_ap, dve_of_8):
                """PSUM->SBUF eviction on ACT or DVE (round-robin)."""
                use_dve = (evict_i[0] % 8) < dve_of_8
                evict_i[0] += 1
                if relu:
                    if use_dve and bias_ap is None:
                        nc.vector.scalar_tensor_tensor(
                            out_ap, ps_ap, 0.0,
                            zcol_s[:, 0:1].to_broadcast(
                                [out_ap.shape[0], out_ap.free_size()]),
                            op0=ALU.max, op1=ALU.add)
                    elif use_dve:
                        # relu(x + b) == max(x, -b) + b ; bias_ap=(be, neg_be)
                        be, nbe = bias_ap
                        nc.vector.scalar_tensor_tensor(
                            out_ap, ps_ap, nbe,
                            be.to_broadcast([out_ap.shape[0], out_ap.free_size()]),
                            op0=ALU.max, op1=ALU.add)
                    else:
                        nc.scalar.activation(out_ap, ps_ap, AF.Relu,
                                             bias=(bias_ap[0] if bias_ap else 0.0))
                else:
                    if use_dve:
                        nc.vector.tensor_copy(out_ap, ps_ap)
                    else:
                        nc.scalar.copy(out_ap, ps_ap)

            loop_ctx = (tc.For_i(0, reps, 1) if hw_loop else contextlib.nullcontext())
            rep_range = range(1 if hw_loop else reps)
            with loop_ctx:
             for rep in rep_range:
              if hw_loop or rep > 0:
                for c in range(2):
                    for t in range(nt):
                        sl = slice(t * nts, (t + 1) * nts)
                        nc.sync.dma_start(hT[c][:, sl], h0T_d[c][:, sl])
                        nc.gpsimd.tensor_copy(hbf[c][:, sl], hT[c][:, sl])
              for l in range(L):
                for i in range(min(4, nb)):
                    nc.gpsimd.tensor_copy(
                        comb_tiles[l][i][64:128, :],
                        w1dp_s[64:128, l * 256:(l + 1) * 256])
                for g in range(nb // 4):
                    agg_ps = psaggp.tile([128, 256], f32, tag="agg",
                                         name=f"agg{l}_{g}")
                    # ---- pass A: hA/hB for 4 blocks
                    for bi in range(4):
                        b = g * 4 + bi
                        ps_ab = psp.tile([128, 512], f32, tag="ps",
                                         name=f"ab{l}_{b}")
                        for kc in range(2):
                            for sel, pos0 in ((0, 0), (1, 32)):
                                nc.tensor.matmul(
                                    ps_ab[pos0:pos0 + 32, 0:256],
                                    lhsT=hbf[kc][:, b * NPB:(b + 1) * NPB],
                                    rhs=w1ab_ap(l, sel, kc),
                                    start=(kc == 0), stop=(kc == 1),
                                    tile_position=(0, pos0),
                                    skip_group_check=True)
                        comb = comb_tiles[l][b % 4]
                        evict(comb[0:64, :], ps_ab[0:64, 0:256], False, None,
                              comb_dve_of_8)
                    # ---- pass B: edge MLP layer 1 (transposed out)
                    m1ts = {}
                    for bi in range(4):
                        b = g * 4 + bi
                        comb = comb_tiles[l][b % 4]
                        m1t = [m1p.tile([128, EPB], bf16, tag=f"m1_{kc}",
                                        name=f"m1_{l}_{b}_{kc}") for kc in range(2)]
                        m1ts[bi] = m1t
                        for mc in range(2):
                            for h in range(2):
                                ps1 = psp.tile([128, 512], f32, tag="ps",
                                               name=f"p1_{l}_{b}_{mc}_{h}")
                                nc.tensor.matmul(
                                    ps1[:, 0:HALF],
                                    lhsT=comb[:, mc * 128:(mc + 1) * 128],
                                    rhs=crg_s[b][:, h * HALF:(h + 1) * HALF],
                                    start=True, stop=True)
                                evict(m1t[mc][:, h * HALF:(h + 1) * HALF],
                                      ps1[:, 0:HALF], True,
                                      (be_ap(0, l, mc), be_ap(3, l, mc)),
                                      m1_dve_of_8)
                    # ---- pass C: edge MLP layer 2 (normal out)
                    m2ss = {}
                    for bi in range(4):
                        b = g * 4 + bi
                        m1t = m1ts[bi]
                        m2sbs = []
                        for p in range(3):
                            ecs = (2 * p, 2 * p + 1) if p < 2 else (4,)
                            w = 256 * len(ecs)
                            ps2 = psp.tile([128, 512], f32, tag="ps",
                                           name=f"p2_{l}_{b}_{p}")
                            for j, ec in enumerate(ecs):
                                if be2_mm:
                                    nc.tensor.matmul(
                                        ps2[:, j * 256:(j + 1) * 256],
                                        lhsT=ones_s[0:1, 0:128],
                                        rhs=be2row_s[0:1, l * 512:l * 512 + 256],
                                        start=True, stop=False)
                                for kc in range(2):
                                    nc.tensor.matmul(
                                        ps2[:, j * 256:(j + 1) * 256],
                                        lhsT=m1t[kc][:, ec * 128:(ec + 1) * 128],
                                        rhs=we2r_ap(l, kc),
                                        start=(kc == 0 and not be2_mm),
                                        stop=(kc == 1))
                            m2sb = m2p.tile([128, 512], bf16, tag="m2",
                                            name=f"m2_{l}_{b}_{p}")
                            evict(m2sb[:, 0:w], ps2[:, 0:w], True, None,
                                  m2_dve_of_8)
                            m2sbs.append(m2sb)
                        m2ss[bi] = m2sbs
                    # ---- pass D: PE segment-sum into agg psum
                    for bi in range(4):
                        m2sbs = m2ss[bi]
                        for mc in range(2):
                            for ec in range(5):
                                p, j = divmod(ec, 2)
                                nc.tensor.matmul(
                                    agg_ps[:, mc * 128 + bi * 32:
                                           mc * 128 + bi * 32 + 32],
                                    lhsT=m2sbs[p][:, j * 256 + mc * 128:
                                                  j * 256 + (mc + 1) * 128],
                                    rhs=ssel_s[:, ec * NPB:(ec + 1) * NPB],
                                    start=(ec == 0), stop=(ec == 4))
                    # ---- agg eviction for this 4-block group (bf16 cast)
                    for mc in range(2):
                        evict(aggbf[mc][:, g * 128:(g + 1) * 128],
                              agg_ps[:, mc * 128:(mc + 1) * 128], False, None,
                              agg_dve_of_8)
                # ---------------- node phase
                for mc in range(2):
                    for t in range(nt):
                        sl = slice(t * nts, (t + 1) * nts)
                        psn = psp.tile([128, 512], f32, tag="ps",
                                       name=f"n1_{l}_{mc}_{t}")
                        nc.tensor.matmul(psn[:, 0:nts], lhsT=bnrow_ap(l, 0, mc),
                                         rhs=ones_s[0:1, 0:nts],
                                         start=True, stop=False)
                        for kc in range(2):
                            nc.tensor.matmul(psn[:, 0:nts],
                                             lhsT=wfam_ap(wn1a_s, l, kc, mc),
                                             rhs=hbf[kc][:, sl],
                                             start=False, stop=False)
                            nc.tensor.matmul(psn[:, 0:nts],
                                             lhsT=wfam_ap(wn1b_s, l, kc, mc),
                                             rhs=aggbf[kc][:, sl],
                                             start=False, stop=(kc == 1))
                        nc.scalar.activation(n1bf[mc][:, sl], psn[:, 0:nts], AF.Relu)
                for mc in range(2):
                    for t in range(nt):
                        sl = slice(t * nts, (t + 1) * nts)
                        pso = psp.tile([128, 512], f32, tag="ps",
                                       name=f"n2_{l}_{mc}_{t}")
                        nc.tensor.matmul(pso[:, 0:nts], lhsT=bnrow_ap(l, 1, mc),
                                         rhs=ones_s[0:1, 0:nts],
                                         start=True, stop=False)
                        for kc in range(2):
                            nc.tensor.matmul(pso[:, 0:nts],
                                             lhsT=wfam_ap(wn2_s, l, kc, mc),
                                             rhs=n1bf[kc][:, sl],
                                             start=False, stop=(kc == 1))
                        nc.vector.scalar_tensor_tensor(
                            hT[mc][:, sl], hT[mc][:, sl], 2.0, pso[:, 0:nts],
                            op0=ALU.mult, op1=ALU.add)
                        if l + 1 < L:
                            nc.gpsimd.tensor_copy(hbf[mc][:, sl], hT[mc][:, sl])
              # ---------------- pooling
              for mc in range(2):
                pool_t = stp.tile([128, nb], f32, tag=f"pool{mc}", name=f"pool{mc}")
                nc.vector.tensor_reduce(pool_t[:],
                                        hT[mc][:].rearrange("p (n k) -> p n k", k=NPB),
                                        axis=mybir.AxisListType.X, op=ALU.add)
                nc.scalar.mul(pool_t[:], pool_t[:], 1.0 / NPB)
                nc.sync.dma_start(out_d[mc], pool_t[:])
    return nc


# ===================================================================== entry

_CACHE = {}


def _get_runner(be2_mm):
    key = ("runner", be2_mm)
    if key not in _CACHE:
        apply_tilefix()
        nc = build_nc_v2(nb=BPC, be2_mm=be2_mm,
                         m1_dve_of_8=4, m2_dve_of_8=4,
                         comb_dve_of_8=5, agg_dve_of_8=2)
        split_waits(nc, cap=1, cap_sp=1)
        _CACHE[key] = nc
    return _CACHE[key]


def kernel(**inputs):
    """Full inputs in (as in reference.setup_inputs), full [B, 256] f32 out."""
    np_inputs = {k: np.asarray(v) for k, v in inputs.items()}
    per_core = host_prep(**np_inputs)
    be2_mm = bool(per_core[0]["be2_nonzero"][0])
    nc = _get_runner(be2_mm)

    import concourse.mybir as mybir
    from concourse.bass_utils import run_bass_kernel_spmd
    declared = set()
    for alloc in nc.m.functions[0].allocations:
        if isinstance(alloc, mybir.MemoryLocationSet) and alloc.kind == "ExternalInput":
            declared.add(alloc.memorylocations[0].name)
    in_maps = [{k: v for k, v in m.items() if k in declared} for m in per_core]
    res = run_bass_kernel_spmd(nc, in_maps, core_ids=list(range(NCORES)))
    return host_unshard(res.results).astype(np.float32)



# revision 4
# speedup vs baseline: 11.4941x; 11.4941x over previous
"""BuildingBlockEmbedder GNN kernel for trn2 — shared library.

Layout: feature-on-partition ("transposed") everywhere on device.
Per core: 64 building blocks x 32 atoms = 2048 nodes, 40960 edges.
"""
import numpy as np
import ml_dtypes

BF16 = ml_dtypes.bfloat16

# problem constants
NUM_GAUSS = 64
MAX_R = 5.0
L = 4
C = 256            # node/hidden channels
NPB = 32           # atoms per block
K = 20             # neighbors
B = 512            # blocks
N = B * NPB
E = N * K
GAUSS_COEFF = -0.5 / (MAX_R / (NUM_GAUSS - 1)) ** 2
NCORES = 8
BPC = B // NCORES          # 64 blocks per core
NPC = BPC * NPB            # 2048 nodes per core
EPB = NPB * K              # 640 edges per block
HALF = EPB // 2            # 320-edge matmul unit

# ---------------------------------------------------------------- host prep

def host_prep(local_coords, atom_types, edge_index, batch_bb, atom_embed, offset,
              We1, be1, We2, be2, Wn1, bn1, Wn2, bn2):
    """Build per-core device inputs from full problem inputs (all numpy)."""
    pos = np.asarray(local_coords, np.float32)
    types = np.asarray(atom_types).astype(np.int64)
    ei = np.asarray(edge_index).astype(np.int64)
    We1 = np.asarray(We1, np.float32); be1 = np.asarray(be1, np.float32)
    We2 = np.asarray(We2, np.float32); be2 = np.asarray(be2, np.float32)
    Wn1 = np.asarray(Wn1, np.float32); bn1 = np.asarray(bn1, np.float32)
    Wn2 = np.asarray(Wn2, np.float32); bn2 = np.asarray(bn2, np.float32)
    emb = np.asarray(atom_embed, np.float32)

    row, col = ei[0], ei[1]
    # structural assumptions from the reference graph builder
    assert np.array_equal(row, np.repeat(np.arange(N, dtype=np.int64), K)), \
        "edge rows must be repeat(arange(N), K)"
    assert np.all(col // NPB == row // NPB), "edges must stay within blocks"

    dvec = pos[col] - pos[row]
    d = np.sqrt((dvec * dvec).sum(-1))          # [E] Angstrom
    assert d.max() < MAX_R - 0.55, f"d.max()={d.max()}: last gaussian not negligible"
    radial = (0.01 * d * d).astype(np.float32)  # ANG_TO_NM^2 * d^2
    # gaussians 0..62 (63rd is exp(<-30) ~= 0 for all d here; its row carries radial)
    off = np.asarray(offset, np.float32)
    gauss = np.exp(GAUSS_COEFF * (d[:, None] - off[None, :63]) ** 2).astype(np.float32)

    # CRG [B, 128, EPB]: rows 0-31 C_sel, 32-63 R_sel, 64 radial, 65-127 gauss
    col_local = (col - (row // NPB) * NPB).astype(np.int32).reshape(B, EPB)
    crg = np.zeros((B, 128, EPB), np.float32)
    e_ar = np.arange(EPB)
    r_sel = np.zeros((NPB, EPB), np.float32)
    r_sel[e_ar // K, e_ar] = 1.0
    for b in range(B):
        crg[b, col_local[b], e_ar] = 1.0      # C_sel
    crg[:, 32:64, :] = r_sel[None]
    crg[:, 64, :] = radial.reshape(B, EPB)
    crg[:, 65:, :] = gauss.reshape(B, EPB, 63).transpose(0, 2, 1)
    crg = crg.astype(BF16)

    h0 = emb[types - 1]                        # [N, C] f32
    h0T = h0.reshape(NCORES, NPC, C).transpose(0, 2, 1).reshape(
        NCORES, 2, 128, NPC).copy()            # [core, chunk, 128, 2048]

    def chunks_lhsT(w):   # w [L, 256, 256] -> [L, kc, mc, 128, 128] bf16
        return np.ascontiguousarray(
            w.reshape(L, 2, 128, 2, 128).transpose(0, 1, 3, 2, 4)).astype(BF16)

    w1b = np.ascontiguousarray(
        We1[:, 256:512, :].reshape(L, 2, 128, 256))      # rhs [L, kc, 128, 256]
    w1a = np.ascontiguousarray(We1[:, 0:256, :].reshape(L, 2, 128, 256))
    w1ab = np.stack([w1b, w1a], axis=1).astype(BF16)     # [L, 2(b,a), kc, 128, 256]
    w1dp = np.concatenate([We1[:, 512:513, :], We1[:, 513:576, :]],
                          axis=1).astype(BF16)           # [L, 64, 256]
    we2 = chunks_lhsT(We2)
    wn1a = chunks_lhsT(Wn1[:, 0:256, :])
    wn1b = chunks_lhsT(Wn1[:, 256:512, :])
    wn2 = chunks_lhsT(Wn2)
    # bias K=1 lhsT rows [L, 2(bn1,bn2), mc, 1, 128] bf16
    bnrow = np.stack([bn1, bn2], axis=1).reshape(L, 2, 2, 1, 128).astype(BF16)
    # ACT/DVE bias columns [128, 4*L*2]; col = (j*L + l)*2 + mc
    # j: 0 = be1, 1 = be2, 2 = -be2, 3 = -be1
    be12 = np.zeros((128, 4 * L * 2), np.float32)
    for j, bb in enumerate([be1, be2, -be2, -be1]):
        for l in range(L):
            for mc in range(2):
                be12[:, (j * L + l) * 2 + mc] = bb[l, mc * 128:(mc + 1) * 128]

    def pmaj(w):  # [L, kc, mc, 128p, 128q] -> [128, L*kc*mc*128]
        return np.ascontiguousarray(w.transpose(3, 0, 1, 2, 4).reshape(128, -1))

    # v2 extras: We2 as rhs [128, L*2*256]; be2 repeated row; Ssel matrices
    we2r = np.ascontiguousarray(
        We2.reshape(L, 2, 128, 256).transpose(2, 0, 1, 3).reshape(128, -1)).astype(BF16)
    be2row = np.ascontiguousarray(
        np.repeat(be2[:, None, :], 2, axis=1).reshape(1, L * 512)).astype(BF16)
    ssel = np.zeros((128, 5 * NPB), np.float32)
    for ec in range(5):
        j = np.arange(128)
        ssel[j, ec * NPB + (ec * 128 + j) // K] = 1.0
    ssel = np.ascontiguousarray(ssel).astype(BF16)

    shared = dict(
        we2r=we2r, be2row=be2row, ssel=ssel,
        be2_nonzero=np.asarray([1.0 if np.any(be2 != 0) else 0.0], np.float32),
        we2=pmaj(we2),
        w1ab=np.ascontiguousarray(w1ab.transpose(3, 0, 1, 2, 4).reshape(128, -1)),
        w1dp=np.ascontiguousarray(w1dp.transpose(1, 0, 2).reshape(64, -1)),
        wn1a=pmaj(wn1a), wn1b=pmaj(wn1b), wn2=pmaj(wn2),
        bnrow=np.ascontiguousarray(bnrow.transpose(3, 0, 1, 2, 4).reshape(1, -1)),
        be12=be12)
    per_core = []
    for c in range(NCORES):
        m = dict(h0T=h0T[c], crg=crg[c * BPC:(c + 1) * BPC])
        m.update(shared)
        per_core.append(m)
    return per_core


def host_unshard(results):
    """results: list of 8 dicts with 'poolT' [2,128,nb] -> full [B, 256] f32."""
    outs = []
    for r in results:
        pt = np.asarray(r["poolT"], np.float32)      # [2, 128, nb]
        nb = pt.shape[2]
        outs.append(pt.reshape(256, nb).T)           # [nb, 256]
    return np.concatenate(outs, axis=0)


# ------------------------------------------------------------ tile drain fix

def apply_tilefix():
    """This container's walrus allows only ONE sem-wait on an SP Drain —
    split the Tile tail-drain waits across serial drains."""
    import concourse.mybir as mybir
    import concourse.tile as tile
    from concourse.tile import ScopedClock

    if getattr(tile.TileContext, "_drain_fix_applied", False):
        return

    def _split(self, tick_clock, wait_clock):
        d = self.nc.sync.drain()
        wait_clock.add_sem_waits(d.ins, ScopedClock({None: tick_clock.global_clock}))
        ws = list(d.ins.sync_info.on_wait) if d.ins.sync_info is not None else []
        if len(ws) > 1:
            d.ins.sync_info.on_wait = ws[:1]
            for w in ws[1:]:
                e = self.nc.sync.drain()
                e.ins.sync_info = mybir.SyncInfo(on_update=[], on_wait=[w])
        self.nc.all_engine_barrier()
        assert self.sems is not None
        popped = self.nc._tile_sem_poison_stack.pop()
        assert popped is self._sem_poison
        self.nc.clear_and_free_semaphores(list(self.sems.allocated().values()))
        self.nc.all_engine_barrier()

    tile.TileContext._drain_and_barrier = _split
    tile.TileContext._drain_fix_applied = True


# ---------------------------------------------------- wait-splitting post-pass

def split_waits(nc, cap=1, cap_sp=1):
    """walrus in this container caps sem-waits per instruction. Hoist excess
    waits onto same-engine NOPs emitted just before the instruction."""
    import concourse.mybir as mybir
    k = 0
    for fn in nc.m.functions:
        for bb in fn.blocks:
            out = []
            for inst in bb.instructions:
                si = inst.sync_info
                ws = list(si.on_wait) if si is not None else []
                c = cap_sp if inst.engine == mybir.EngineType.SP else cap
                if len(ws) > c:
                    keep = ws[:c] if c > 0 else []
                    rest = ws[c:] if c > 0 else ws
                    while rest:
                        chunk, rest = rest[:max(c, 1)], rest[max(c, 1):]
                        nop = mybir.InstNoOp(
                            name=f"wsplit-{k}", engine=inst.engine,
                            sync_info=mybir.SyncInfo(on_wait=chunk, on_update=[]),
                            bass_nofuse=True)
                        k += 1
                        out.append(nop)
                    inst.sync_info.on_wait = keep
                out.append(inst)
            bb.instructions[:] = out
    return k


# ------------------------------------------------------------- bass builder

def build_nc(nb=BPC, reps=1, hw_loop=False):
    """Build the per-core Bass module. nb = blocks per core (small for sim)."""
    import concourse.bass as bass
    import concourse.mybir as mybir
    import concourse.tile as tile

    f32, bf16 = mybir.dt.float32, mybir.dt.bfloat16
    AF = mybir.ActivationFunctionType
    ALU = mybir.AluOpType
    nn = nb * NPB                     # nodes this build
    nts = min(512, nn)                # node tile size
    nt = nn // nts                    # node tiles

    nc = bass.Bass()
    h0T_d = nc.dram_tensor("h0T", [2, 128, nn], f32, kind="ExternalInput")
    crg_d = nc.dram_tensor("crg", [nb, 128, EPB], bf16, kind="ExternalInput")
    we2_d = nc.dram_tensor("we2", [128, L * 2 * 2 * 128], bf16, kind="ExternalInput")
    w1ab_d = nc.dram_tensor("w1ab", [128, L * 2 * 2 * 256], bf16, kind="ExternalInput")
    w1dp_d = nc.dram_tensor("w1dp", [64, L * 256], bf16, kind="ExternalInput")
    wn1a_d = nc.dram_tensor("wn1a", [128, L * 2 * 2 * 128], bf16, kind="ExternalInput")
    wn1b_d = nc.dram_tensor("wn1b", [128, L * 2 * 2 * 128], bf16, kind="ExternalInput")
    wn2_d = nc.dram_tensor("wn2", [128, L * 2 * 2 * 128], bf16, kind="ExternalInput")
    bnrow_d = nc.dram_tensor("bnrow", [1, L * 2 * 2 * 128], bf16, kind="ExternalInput")
    be12_d = nc.dram_tensor("be12", [128, 4 * L * 2], f32, kind="ExternalInput")
    out_d = nc.dram_tensor("poolT", [2, 128, nb], f32, kind="ExternalOutput")

    with tile.TileContext(nc) as tc:
        with (
            tc.tile_pool(name="const", bufs=1) as csp,
            tc.tile_pool(name="crgp", bufs=1) as crgp,
            tc.tile_pool(name="state", bufs=1) as stp,
            tc.tile_pool(name="comb", bufs=4) as combp,
            tc.tile_pool(name="m1p", bufs=3) as m1p,
            tc.tile_pool(name="m2p", bufs=6) as m2p,
            tc.tile_pool(name="ps", bufs=8, space="PSUM") as psp,
        ):
            # ---- resident constants
            we2_s = csp.tile([128, L * 2 * 2 * 128], bf16, name="we2_s")
            nc.sync.dma_start(we2_s[:], we2_d[:])
            w1ab_s = csp.tile([128, L * 2 * 2 * 256], bf16, name="w1ab_s")
            nc.sync.dma_start(w1ab_s[:], w1ab_d[:])
            w1dp_s = csp.tile([128, L * 256], bf16, name="w1dp_s")
            nc.sync.dma_start(w1dp_s[64:128, :], w1dp_d[:])
            wn1a_s = csp.tile([128, L * 2 * 2 * 128], bf16, name="wn1a_s")
            nc.sync.dma_start(wn1a_s[:], wn1a_d[:])
            wn1b_s = csp.tile([128, L * 2 * 2 * 128], bf16, name="wn1b_s")
            nc.sync.dma_start(wn1b_s[:], wn1b_d[:])
            wn2_s = csp.tile([128, L * 2 * 2 * 128], bf16, name="wn2_s")
            nc.sync.dma_start(wn2_s[:], wn2_d[:])
            bnrow_s = csp.tile([128, L * 2 * 2 * 128], bf16, name="bnrow_s")
            nc.sync.dma_start(bnrow_s[0:1, :], bnrow_d[:])
            be12_s = csp.tile([128, 4 * L * 2], f32, name="be12_s")
            nc.sync.dma_start(be12_s[:], be12_d[:])
            ones_s = csp.tile([128, 512], bf16, name="ones_s")
            nc.gpsimd.memset(ones_s[0:1, :], 1.0)

            def we2_ap(l, kc, mc):
                o = ((l * 2 + kc) * 2 + mc) * 128
                return we2_s[:, o:o + 128]

            def w1ab_ap(l, s, kc):
                o = ((l * 2 + s) * 2 + kc) * 256
                return w1ab_s[:, o:o + 256]

            def wfam_ap(t, l, kc, mc):
                o = ((l * 2 + kc) * 2 + mc) * 128
                return t[:, o:o + 128]

            def bnrow_ap(l, j, mc):
                o = ((l * 2 + j) * 2 + mc) * 128
                return bnrow_s[0:1, o:o + 128]

            def be_ap(j, l, mc):
                o = (j * L + l) * 2 + mc
                return be12_s[:, o:o + 1]

            # ---- CRG resident
            crg_s = []
            for b in range(nb):
                t = crgp.tile([128, EPB], bf16, name=f"crg{b}", tag=f"crg{b}")
                nc.sync.dma_start(t[:], crg_d[b])
                crg_s.append(t)

            # ---- state
            hT, hbf, aggT, aggbf, n1bf = [], [], [], [], []
            for c in range(2):
                t = stp.tile([128, nn], f32, name=f"hT{c}", tag=f"hT{c}")
                hT.append(t)
                hbf.append(stp.tile([128, nn], bf16, name=f"hbf{c}", tag=f"hbf{c}"))
                aggT.append(stp.tile([128, nn], f32, name=f"aggT{c}", tag=f"aggT{c}"))
                aggbf.append(stp.tile([128, nn], bf16, name=f"aggbf{c}", tag=f"agb{c}"))
                n1bf.append(stp.tile([128, nn], bf16, name=f"n1bf{c}", tag=f"n1b{c}"))

            import contextlib
            loop_ctx = (tc.For_i(0, reps, 1) if hw_loop
                        else contextlib.nullcontext())
            rep_range = range(1 if hw_loop else reps)
            with loop_ctx:
             for rep in rep_range:
              for c in range(2):
                nc.sync.dma_start(hT[c][:], h0T_d[c])
              for l in range(L):
                for c in range(2):
                    nc.gpsimd.tensor_copy(hbf[c][:], hT[c][:])
                # ---------------- edge phase, per block
                for b in range(nb):
                    ps_ab = psp.tile([128, 512], f32, tag="ps", name=f"ab{l}_{b}")
                    for sel, pos0 in ((0, 0), (1, 32)):
                        for kc in range(2):
                            nc.tensor.matmul(ps_ab[pos0:pos0 + 32, 0:256],
                                             lhsT=hbf[kc][:, b * NPB:(b + 1) * NPB],
                                             rhs=w1ab_ap(l, sel, kc),
                                             start=(kc == 0), stop=(kc == 1),
                                             tile_position=(0, pos0))
                    comb = combp.tile([128, 256], bf16, tag="comb", name=f"cb{l}_{b}")
                    nc.scalar.copy(comb[0:64, :], ps_ab[0:64, 0:256])
                    nc.gpsimd.tensor_copy(comb[64:128, :],
                                          w1dp_s[64:128, l * 256:(l + 1) * 256])
                    m1t = [m1p.tile([128, EPB], bf16, tag=f"m1_{kc}",
                                    name=f"m1_{l}_{b}_{kc}") for kc in range(2)]
                    for mc in range(2):
                        for h in range(2):
                            ps1 = psp.tile([128, 512], f32, tag="ps",
                                           name=f"p1_{l}_{b}_{mc}_{h}")
                            nc.tensor.matmul(ps1[:, 0:HALF],
                                             lhsT=comb[:, mc * 128:(mc + 1) * 128],
                                             rhs=crg_s[b][:, h * HALF:(h + 1) * HALF],
                                             start=True, stop=True)
                            nc.scalar.activation(m1t[mc][:, h * HALF:(h + 1) * HALF],
                                                 ps1[:, 0:HALF], AF.Relu,
                                                 bias=be_ap(0, l, mc))
                    for mc in range(2):
                        for h in range(2):
                            ps2 = psp.tile([128, 512], f32, tag="ps",
                                           name=f"p2_{l}_{b}_{mc}_{h}")
                            for kc in range(2):
                                nc.tensor.matmul(
                                    ps2[:, 0:HALF],
                                    lhsT=we2_ap(l, kc, mc),
                                    rhs=m1t[kc][:, h * HALF:(h + 1) * HALF],
                                    start=(kc == 0), stop=(kc == 1))
                            m2t = m2p.tile([128, HALF], bf16, tag="m2",
                                           name=f"m2_{l}_{b}_{mc}_{h}")
                            if (b * 4 + mc * 2 + h) % 2:
                                nc.scalar.activation(m2t[:], ps2[:, 0:HALF], AF.Relu,
                                                     bias=be_ap(1, l, mc))
                            else:
                                # relu(x + b) == max(x, -b) + b
                                nc.vector.scalar_tensor_tensor(
                                    m2t[:], ps2[:, 0:HALF], be_ap(2, l, mc),
                                    be_ap(1, l, mc).to_broadcast([128, HALF]),
                                    op0=ALU.max, op1=ALU.add)
                            nc.vector.tensor_reduce(
                                aggT[mc][:, b * NPB + h * 16: b * NPB + (h + 1) * 16],
                                m2t[:].rearrange("p (n k) -> p n k", k=K),
                                axis=mybir.AxisListType.X, op=ALU.add)
                # ---------------- node phase
                for c in range(2):
                    nc.gpsimd.tensor_copy(aggbf[c][:], aggT[c][:])
                for mc in range(2):
                    for t in range(nt):
                        sl = slice(t * nts, (t + 1) * nts)
                        psn = psp.tile([128, 512], f32, tag="ps",
                                       name=f"n1_{l}_{mc}_{t}")
                        nc.tensor.matmul(psn[:, 0:nts], lhsT=bnrow_ap(l, 0, mc),
                                         rhs=ones_s[0:1, 0:nts], start=True, stop=False)
                        for kc in range(2):
                            nc.tensor.matmul(psn[:, 0:nts],
                                             lhsT=wfam_ap(wn1a_s, l, kc, mc),
                                             rhs=hbf[kc][:, sl], start=False, stop=False)
                            nc.tensor.matmul(psn[:, 0:nts],
                                             lhsT=wfam_ap(wn1b_s, l, kc, mc),
                                             rhs=aggbf[kc][:, sl], start=False,
                                             stop=(kc == 1))
                        nc.scalar.activation(n1bf[mc][:, sl], psn[:, 0:nts], AF.Relu)
                for mc in range(2):
                    for t in range(nt):
                        sl = slice(t * nts, (t + 1) * nts)
                        pso = psp.tile([128, 512], f32, tag="ps",
                                       name=f"n2_{l}_{mc}_{t}")
                        nc.tensor.matmul(pso[:, 0:nts], lhsT=bnrow_ap(l, 1, mc),
                                         rhs=ones_s[0:1, 0:nts], start=True, stop=False)
                        for kc in range(2):
                            nc.tensor.matmul(pso[:, 0:nts],
                                             lhsT=wfam_ap(wn2_s, l, kc, mc),
                                             rhs=n1bf[kc][:, sl], start=False,
                                             stop=(kc == 1))
                        nc.vector.scalar_tensor_tensor(
                            hT[mc][:, sl], hT[mc][:, sl], 2.0, pso[:, 0:nts],
                            op0=ALU.mult, op1=ALU.add)
              # ---------------- pooling
              for mc in range(2):
                pool_t = stp.tile([128, nb], f32, tag=f"pool{mc}", name=f"pool{mc}")
                nc.vector.tensor_reduce(pool_t[:],
                                        hT[mc][:].rearrange("p (n k) -> p n k", k=NPB),
                                        axis=mybir.AxisListType.X, op=ALU.add)
                nc.scalar.mul(pool_t[:], pool_t[:], 1.0 / NPB)
                nc.sync.dma_start(out_d[mc], pool_t[:])
    return nc


# --------------------------------------------------- numpy model of the math

def numpy_model(ins, nb=BPC, cores=None):
    """Replicate the device math (incl. bf16 rounding) for validation.
    ins: list of per-core input dicts (from host_prep). Returns [sum_nb*NCORES? , 256]."""
    outs = []
    for m in (ins if cores is None else [ins[c] for c in cores]):
        h = np.asarray(m["h0T"], np.float32).reshape(256, -1)[:, :nb * NPB]  # [256, nn]
        crg = np.asarray(m["crg"], np.float32)[:nb]
        L4 = L
        we2 = np.asarray(m["we2"], np.float32).reshape(128, L4, 2, 2, 128).transpose(1, 2, 3, 0, 4)
        w1ab = np.asarray(m["w1ab"], np.float32).reshape(128, L4, 2, 2, 256).transpose(1, 2, 3, 0, 4)
        w1dp = np.asarray(m["w1dp"], np.float32).reshape(64, L4, 256).transpose(1, 0, 2)
        wn1a = np.asarray(m["wn1a"], np.float32).reshape(128, L4, 2, 2, 128).transpose(1, 2, 3, 0, 4)
        wn1b = np.asarray(m["wn1b"], np.float32).reshape(128, L4, 2, 2, 128).transpose(1, 2, 3, 0, 4)
        wn2 = np.asarray(m["wn2"], np.float32).reshape(128, L4, 2, 2, 128).transpose(1, 2, 3, 0, 4)
        bnrow = np.asarray(m["bnrow"], np.float32).reshape(1, L4, 2, 2, 128).transpose(1, 2, 3, 0, 4)
        be12 = np.asarray(m["be12"], np.float32)
        nn = nb * NPB

        def b16(x):
            return x.astype(BF16).astype(np.float32)

        def blk(w):  # [kc, mc, 128, 128] -> [256, 256]
            return np.concatenate(
                [np.concatenate([w[kc_, mc_] for mc_ in range(2)], axis=1)
                 for kc_ in range(2)], axis=0)

        for l in range(L):
            hb = b16(h)                                    # [256, nn]
            # hAB per block
            W1b = np.concatenate([w1ab[l, 0, kc_] for kc_ in range(2)], axis=0)
            W1a = np.concatenate([w1ab[l, 1, kc_] for kc_ in range(2)], axis=0)
            be1 = np.concatenate([be12[:, (0 * L + l) * 2 + mc_] for mc_ in range(2)])
            be2 = np.concatenate([be12[:, (1 * L + l) * 2 + mc_] for mc_ in range(2)])
            agg = np.zeros((256, nn), np.float32)
            for b in range(nb):
                hs = hb[:, b * NPB:(b + 1) * NPB]          # [256, 32]
                hB = b16(hs.T @ W1b)                       # [32, 256] evicted bf16
                hA = b16(hs.T @ W1a)
                combined = np.concatenate([hB, hA, w1dp[l]], axis=0)  # [128, 256]
                pre1 = combined.T @ crg[b]                 # [256, EPB]
                m1 = b16(np.maximum(pre1 + be1[:, None], 0.0))
                W2 = blk(we2[l])
                m2 = b16(np.maximum(W2.T @ m1 + be2[:, None], 0.0))
                agg[:, b * NPB:(b + 1) * NPB] = (
                    m2.reshape(256, NPB, K).sum(axis=2))
            aggb = b16(agg)
            N1a, N1b_, N2 = blk(wn1a[l]), blk(wn1b[l]), blk(wn2[l])
            bn1 = bnrow[l, 0].reshape(256)
            bn2 = bnrow[l, 1].reshape(256)
            n1 = b16(np.maximum(N1a.T @ hb + N1b_.T @ aggb + bn1[:, None], 0.0))
            out = N2.T @ n1 + bn2[:, None]
            h = 2.0 * h + out
        pooled = h.reshape(256, nb, NPB).mean(axis=2)       # [256, nb]
        outs.append(pooled.T)
    return np.concatenate(outs, axis=0)


# --------------------------------------------------------------- builder v2
# m2 in normal layout (edges on partitions); segment-sum as PE matmuls with
# constant Ssel matrices; agg evicted straight to bf16.

def build_nc_v2(nb=BPC, reps=1, hw_loop=False, be2_mm=False,
                m1_dve_of_8=2, m2_dve_of_8=5, comb_dve_of_8=0, agg_dve_of_8=0):
    import contextlib
    import concourse.bass as bass
    import concourse.mybir as mybir
    import concourse.tile as tile

    f32, bf16 = mybir.dt.float32, mybir.dt.bfloat16
    AF = mybir.ActivationFunctionType
    ALU = mybir.AluOpType
    nn = nb * NPB
    nts = min(512, nn)
    nt = nn // nts

    nc = bass.Bass()
    h0T_d = nc.dram_tensor("h0T", [2, 128, nn], f32, kind="ExternalInput")
    crg_d = nc.dram_tensor("crg", [nb, 128, EPB], bf16, kind="ExternalInput")
    we2r_d = nc.dram_tensor("we2r", [128, L * 2 * 256], bf16, kind="ExternalInput")
    w1ab_d = nc.dram_tensor("w1ab", [128, L * 2 * 2 * 256], bf16, kind="ExternalInput")
    w1dp_d = nc.dram_tensor("w1dp", [64, L * 256], bf16, kind="ExternalInput")
    wn1a_d = nc.dram_tensor("wn1a", [128, L * 2 * 2 * 128], bf16, kind="ExternalInput")
    wn1b_d = nc.dram_tensor("wn1b", [128, L * 2 * 2 * 128], bf16, kind="ExternalInput")
    wn2_d = nc.dram_tensor("wn2", [128, L * 2 * 2 * 128], bf16, kind="ExternalInput")
    bnrow_d = nc.dram_tensor("bnrow", [1, L * 2 * 2 * 128], bf16, kind="ExternalInput")
    be12_d = nc.dram_tensor("be12", [128, 4 * L * 2], f32, kind="ExternalInput")
    be2row_d = nc.dram_tensor("be2row", [1, L * 512], bf16, kind="ExternalInput")
    ssel_d = nc.dram_tensor("ssel", [128, 5 * NPB], bf16, kind="ExternalInput")
    out_d = nc.dram_tensor("poolT", [2, 128, nb], f32, kind="ExternalOutput")

    with tile.TileContext(nc) as tc:
        with (
            tc.tile_pool(name="const", bufs=1) as csp,
            tc.tile_pool(name="crgp", bufs=1) as crgp,
            tc.tile_pool(name="state", bufs=1) as stp,
            tc.tile_pool(name="comb", bufs=1) as combp,
            tc.tile_pool(name="m1p", bufs=5) as m1p,
            tc.tile_pool(name="m2p", bufs=14) as m2p,
            tc.tile_pool(name="ps", bufs=7, space="PSUM") as psp,
            tc.tile_pool(name="psagg", bufs=1, space="PSUM") as psaggp,
        ):
            we2r_s = csp.tile([128, L * 2 * 256], bf16, name="we2r_s")
            nc.sync.dma_start(we2r_s[:], we2r_d[:])
            w1ab_s = csp.tile([128, L * 2 * 2 * 256], bf16, name="w1ab_s")
            nc.sync.dma_start(w1ab_s[:], w1ab_d[:])
            w1dp_s = csp.tile([128, L * 256], bf16, name="w1dp_s")
            nc.sync.dma_start(w1dp_s[64:128, :], w1dp_d[:])
            wn1a_s = csp.tile([128, L * 2 * 2 * 128], bf16, name="wn1a_s")
            nc.sync.dma_start(wn1a_s[:], wn1a_d[:])
            wn1b_s = csp.tile([128, L * 2 * 2 * 128], bf16, name="wn1b_s")
            nc.sync.dma_start(wn1b_s[:], wn1b_d[:])
            wn2_s = csp.tile([128, L * 2 * 2 * 128], bf16, name="wn2_s")
            nc.sync.dma_start(wn2_s[:], wn2_d[:])
            bnrow_s = csp.tile([128, L * 2 * 2 * 128], bf16, name="bnrow_s")
            nc.sync.dma_start(bnrow_s[0:1, :], bnrow_d[:])
            be12_s = csp.tile([128, 4 * L * 2], f32, name="be12_s")
            nc.sync.dma_start(be12_s[:], be12_d[:])
            be2row_s = csp.tile([128, L * 512], bf16, name="be2row_s")
            nc.sync.dma_start(be2row_s[0:1, :], be2row_d[:])
            ssel_s = csp.tile([128, 5 * NPB], bf16, name="ssel_s")
            nc.sync.dma_start(ssel_s[:], ssel_d[:])
            ones_s = csp.tile([128, 512], bf16, name="ones_s")
            nc.gpsimd.memset(ones_s[0:1, :], 1.0)
            zcol_s = csp.tile([128, 1], f32, name="zcol_s")
            nc.gpsimd.memset(zcol_s[:], 0.0)

            def we2r_ap(l, kc):
                o = (l * 2 + kc) * 256
                return we2r_s[:, o:o + 256]

            def w1ab_ap(l, sel, kc):
                o = ((l * 2 + sel) * 2 + kc) * 256
                return w1ab_s[:, o:o + 256]

            def wfam_ap(t, l, kc, mc):
                o = ((l * 2 + kc) * 2 + mc) * 128
                return t[:, o:o + 128]

            def bnrow_ap(l, j, mc):
                o = ((l * 2 + j) * 2 + mc) * 128
                return bnrow_s[0:1, o:o + 128]

            def be_ap(j, l, mc):
                o = (j * L + l) * 2 + mc
                return be12_s[:, o:o + 1]

            hT, hbf, aggbf, n1bf = [], [], [], []
            for c in range(2):
                hT.append(stp.tile([128, nn], f32, name=f"hT{c}", tag=f"hT{c}"))
                hbf.append(stp.tile([128, nn], bf16, name=f"hbf{c}", tag=f"hbf{c}"))
                aggbf.append(stp.tile([128, nn], bf16, name=f"agb{c}", tag=f"agb{c}"))
                n1bf.append(stp.tile([128, nn], bf16, name=f"n1b{c}", tag=f"n1b{c}"))

            if not hw_loop:
                for c in range(2):
                    for t in range(nt):
                        sl = slice(t * nts, (t + 1) * nts)
                        nc.sync.dma_start(hT[c][:, sl], h0T_d[c][:, sl])
                        nc.gpsimd.tensor_copy(hbf[c][:, sl], hT[c][:, sl])

            crg_s = []
            for b in range(nb):
                t = crgp.tile([128, EPB], bf16, name=f"crg{b}", tag=f"crg{b}")
                nc.sync.dma_start(t[:], crg_d[b])
                crg_s.append(t)


            comb_tiles = [
                [combp.tile([128, 256], bf16, tag=f"comb{l}_{i}",
                            name=f"comb{l}_{i}") for i in range(min(4, nb))]
                for l in range(L)]

            evict_i = [0]

            def evict(out_ap, ps_ap, relu, bias_ap, dve_of_8):
                """PSUM->SBUF eviction on ACT or DVE (round-robin)."""
                use_dve = (evict_i[0] % 8) < dve_of_8
                evict_i[0] += 1
                if relu:
                    if use_dve and bias_ap is None:
                        nc.vector.scalar_tensor_tensor(
                            out_ap, ps_ap, 0.0,
                            zcol_s[:, 0:1].to_broadcast(
                                [out_ap.shape[0], out_ap.free_size()]),
                            op0=ALU.max, op1=ALU.add)
                    elif use_dve:
                        # relu(x + b) == max(x, -b) + b ; bias_ap=(be, neg_be)
                        be, nbe = bias_ap
                        nc.vector.scalar_tensor_tensor(
                            out_ap, ps_ap, nbe,
                            be.to_broadcast([out_ap.shape[0], out_ap.free_size()]),
                            op0=ALU.max, op1=ALU.add)
                    else:
                        nc.scalar.activation(out_ap, ps_ap, AF.Relu,
                                             bias=(bias_ap[0] if bias_ap else 0.0))
                else:
                    if use_dve:
                        nc.vector.tensor_copy(out_ap, ps_ap)
                    else:
                        nc.scalar.copy(out_ap, ps_ap)

            loop_ctx = (tc.For_i(0, reps, 1) if hw_loop else contextlib.nullcontext())
            rep_range = range(1 if hw_loop else reps)
            with loop_ctx:
             for rep in rep_range:
              if hw_loop or rep > 0:
                for c in range(2):
                    for t in range(nt):
                        sl = slice(t * nts, (t + 1) * nts)
                        nc.sync.dma_start(hT[c][:, sl], h0T_d[c][:, sl])
                        nc.gpsimd.tensor_copy(hbf[c][:, sl], hT[c][:, sl])
              for l in range(L):
                for i in range(min(4, nb)):
                    nc.gpsimd.tensor_copy(
                        comb_tiles[l][i][64:128, :],
                        w1dp_s[64:128, l * 256:(l + 1) * 256])
                for g in range(nb // 4):
                    agg_ps = psaggp.tile([128, 256], f32, tag="agg",
                                         name=f"agg{l}_{g}")
                    # ---- pass A: hA/hB for 4 blocks
                    for bi in range(4):
                        b = g * 4 + bi
                        ps_ab = psp.tile([128, 512], f32, tag="ps",
                                         name=f"ab{l}_{b}")
                        for kc in range(2):
                            for sel, pos0 in ((0, 0), (1, 32)):
                                nc.tensor.matmul(
                                    ps_ab[pos0:pos0 + 32, 0:256],
                                    lhsT=hbf[kc][:, b * NPB:(b + 1) * NPB],
                                    rhs=w1ab_ap(l, sel, kc),
                                    start=(kc == 0), stop=(kc == 1),
                                    tile_position=(0, pos0),
                                    skip_group_check=True)
                        comb = comb_tiles[l][b % 4]
                        evict(comb[0:64, :], ps_ab[0:64, 0:256], False, None,
                              comb_dve_of_8)
                    # ---- pass B: edge MLP layer 1 (transposed out)
                    m1ts = {}
                    for bi in range(4):
                        b = g * 4 + bi
                        comb = comb_tiles[l][b % 4]
                        m1t = [m1p.tile([128, EPB], bf16, tag=f"m1_{kc}",
                                        name=f"m1_{l}_{b}_{kc}") for kc in range(2)]
                        m1ts[bi] = m1t
                        for mc in range(2):
                            for h in range(2):
                                ps1 = psp.tile([128, 512], f32, tag="ps",
                                               name=f"p1_{l}_{b}_{mc}_{h}")
                                nc.tensor.matmul(
                                    ps1[:, 0:HALF],
                                    lhsT=comb[:, mc * 128:(mc + 1) * 128],
                                    rhs=crg_s[b][:, h * HALF:(h + 1) * HALF],
                                    start=True, stop=True)
                                evict(m1t[mc][:, h * HALF:(h + 1) * HALF],
                                      ps1[:, 0:HALF], True,
                                      (be_ap(0, l, mc), be_ap(3, l, mc)),
                                      m1_dve_of_8)
                    # ---- pass C: edge MLP layer 2 (normal out)
                    m2ss = {}
                    for bi in range(4):
                        b = g * 4 + bi
                        m1t = m1ts[bi]
                        m2sbs = []
                        for p in range(3):
                            ecs = (2 * p, 2 * p + 1) if p < 2 else (4,)
                            w = 256 * len(ecs)
                            ps2 = psp.tile([128, 512], f32, tag="ps",
                                           name=f"p2_{l}_{b}_{p}")
                            for j, ec in enumerate(ecs):
                                if be2_mm:
                                    nc.tensor.matmul(
                                        ps2[:, j * 256:(j + 1) * 256],
                                        lhsT=ones_s[0:1, 0:128],
                                        rhs=be2row_s[0:1, l * 512:l * 512 + 256],
                                        start=True, stop=False)
                                for kc in range(2):
                                    nc.tensor.matmul(
                                        ps2[:, j * 256:(j + 1) * 256],
                                        lhsT=m1t[kc][:, ec * 128:(ec + 1) * 128],
                                        rhs=we2r_ap(l, kc),
                                        start=(kc == 0 and not be2_mm),
                                        stop=(kc == 1))
                            m2sb = m2p.tile([128, 512], bf16, tag="m2",
                                            name=f"m2_{l}_{b}_{p}")
                            evict(m2sb[:, 0:w], ps2[:, 0:w], True, None,
                                  m2_dve_of_8)
                            m2sbs.append(m2sb)
                        m2ss[bi] = m2sbs
                    # ---- pass D: PE segment-sum into agg psum
                    for bi in range(4):
                        m2sbs = m2ss[bi]
                        for mc in range(2):
                            for ec in range(5):
                                p, j = divmod(ec, 2)
                                nc.tensor.matmul(
                                    agg_ps[:, mc * 128 + bi * 32:
                                           mc * 128 + bi * 32 + 32],
                                    lhsT=m2sbs[p][:, j * 256 + mc * 128:
                                                  j * 256 + (mc + 1) * 128],
                                    rhs=ssel_s[:, ec * NPB:(ec + 1) * NPB],
                                    start=(ec == 0), stop=(ec == 4))
                    # ---- agg eviction for this 4-block group (bf16 cast)
                    for mc in range(2):
                        evict(aggbf[mc][:, g * 128:(g + 1) * 128],
                              agg_ps[:, mc * 128:(mc + 1) * 128], False, None,
                              agg_dve_of_8)
                # ---------------- node phase
                for mc in range(2):
                    for t in range(nt):
                        sl = slice(t * nts, (t + 1) * nts)
                        psn = psp.tile([128, 512], f32, tag="ps",
                                       name=f"n1_{l}_{mc}_{t}")
                        nc.tensor.matmul(psn[:, 0:nts], lhsT=bnrow_ap(l, 0, mc),
                                         rhs=ones_s[0:1, 0:nts],
                                         start=True, stop=False)
                        for kc in range(2):
                            nc.tensor.matmul(psn[:, 0:nts],
                                             lhsT=wfam_ap(wn1a_s, l, kc, mc),
                                             rhs=hbf[kc][:, sl],
                                             start=False, stop=False)
                            nc.tensor.matmul(psn[:, 0:nts],
                                             lhsT=wfam_ap(wn1b_s, l, kc, mc),
                                             rhs=aggbf[kc][:, sl],
                                             start=False, stop=(kc == 1))
                        nc.scalar.activation(n1bf[mc][:, sl], psn[:, 0:nts], AF.Relu)
                for mc in range(2):
                    for t in range(nt):
                        sl = slice(t * nts, (t + 1) * nts)
                        pso = psp.tile([128, 512], f32, tag="ps",
                                       name=f"n2_{l}_{mc}_{t}")
                        nc.tensor.matmul(pso[:, 0:nts], lhsT=bnrow_ap(l, 1, mc),
                                         rhs=ones_s[0:1, 0:nts],
                                         start=True, stop=False)
                        for kc in range(2):
                            nc.tensor.matmul(pso[:, 0:nts],
                                             lhsT=wfam_ap(wn2_s, l, kc, mc),
                                             rhs=n1bf[kc][:, sl],
                                             start=False, stop=(kc == 1))
                        nc.vector.scalar_tensor_tensor(
                            hT[mc][:, sl], hT[mc][:, sl], 2.0, pso[:, 0:nts],
                            op0=ALU.mult, op1=ALU.add)
                        if l + 1 < L:
                            nc.gpsimd.tensor_copy(hbf[mc][:, sl], hT[mc][:, sl])
              # ---------------- pooling
              for mc in range(2):
                pool_t = stp.tile([128, nb], f32, tag=f"pool{mc}", name=f"pool{mc}")
                nc.vector.tensor_reduce(pool_t[:],
                                        hT[mc][:].rearrange("p (n k) -> p n k", k=NPB),
                                        axis=mybir.AxisListType.X, op=ALU.add)
                nc.scalar.mul(pool_t[:], pool_t[:], 1.0 / NPB)
                nc.sync.dma_start(out_d[mc], pool_t[:])
    return nc


# --------------------------------------------------------------- builder v3
# v2 + single bank-crossing evictions for m1/m2 (packed multi-bank psum),
# merged agg eviction, zero-bias node phase, shared 3-bank psum pool.

def build_nc_v3(nb=BPC, reps=1, hw_loop=False, be2_mm=False, bn_zero=True,
                be1_zero=True,
                comb_dve_of_8=4, m1_dve_of_8=4, m2_dve_of_8=4, agg_dve_of_8=4,
                n1_dve_of_8=0):
    import contextlib
    import concourse.bass as bass
    import concourse.mybir as mybir
    import concourse.tile as tile

    f32, bf16 = mybir.dt.float32, mybir.dt.bfloat16
    AF = mybir.ActivationFunctionType
    ALU = mybir.AluOpType
    nn = nb * NPB
    nts = min(512, nn)
    nt = nn // nts

    nc = bass.Bass()
    h0T_d = nc.dram_tensor("h0T", [2, 128, nn], f32, kind="ExternalInput")
    crg_d = nc.dram_tensor("crg", [nb, 128, EPB], bf16, kind="ExternalInput")
    we2r_d = nc.dram_tensor("we2r", [128, L * 2 * 256], bf16, kind="ExternalInput")
    w1ab_d = nc.dram_tensor("w1ab", [128, L * 2 * 2 * 256], bf16, kind="ExternalInput")
    w1dp_d = nc.dram_tensor("w1dp", [64, L * 256], bf16, kind="ExternalInput")
    wn1a_d = nc.dram_tensor("wn1a", [128, L * 2 * 2 * 128], bf16, kind="ExternalInput")
    wn1b_d = nc.dram_tensor("wn1b", [128, L * 2 * 2 * 128], bf16, kind="ExternalInput")
    wn2_d = nc.dram_tensor("wn2", [128, L * 2 * 2 * 128], bf16, kind="ExternalInput")
    be12_d = nc.dram_tensor("be12", [128, 4 * L * 2], f32, kind="ExternalInput")
    ssel_d = nc.dram_tensor("ssel", [128, 5 * NPB], bf16, kind="ExternalInput")
    if not bn_zero:
        bnrow_d = nc.dram_tensor("bnrow", [1, L * 2 * 2 * 128], bf16,
                                 kind="ExternalInput")
    if be2_mm:
        be2row_d = nc.dram_tensor("be2row", [1, L * 512], bf16,
                                  kind="ExternalInput")
    out_d = nc.dram_tensor("poolT", [2, 128, nb], f32, kind="ExternalOutput")

    with tile.TileContext(nc) as tc:
        with (
            tc.tile_pool(name="const", bufs=1) as csp,
            tc.tile_pool(name="crgp", bufs=1) as crgp,
            tc.tile_pool(name="state", bufs=1) as stp,
            tc.tile_pool(name="comb", bufs=1) as combp,
            tc.tile_pool(name="m1p", bufs=3) as m1p,
            tc.tile_pool(name="m2p", bufs=4) as m2p,
            tc.tile_pool(name="big", bufs=2, space="PSUM") as bigp,
            tc.tile_pool(name="psagg", bufs=2, space="PSUM") as psaggp,
        ):
            we2r_s = csp.tile([128, L * 2 * 256], bf16, name="we2r_s")
            nc.sync.dma_start(we2r_s[:], we2r_d[:])
            w1ab_s = csp.tile([128, L * 2 * 2 * 256], bf16, name="w1ab_s")
            nc.sync.dma_start(w1ab_s[:], w1ab_d[:])
            w1dp_s = csp.tile([128, L * 256], bf16, name="w1dp_s")
            nc.sync.dma_start(w1dp_s[64:128, :], w1dp_d[:])
            wn1a_s = csp.tile([128, L * 2 * 2 * 128], bf16, name="wn1a_s")
            nc.sync.dma_start(wn1a_s[:], wn1a_d[:])
            wn1b_s = csp.tile([128, L * 2 * 2 * 128], bf16, name="wn1b_s")
            nc.sync.dma_start(wn1b_s[:], wn1b_d[:])
            wn2_s = csp.tile([128, L * 2 * 2 * 128], bf16, name="wn2_s")
            nc.sync.dma_start(wn2_s[:], wn2_d[:])
            be12_s = csp.tile([128, 4 * L * 2], f32, name="be12_s")
            nc.sync.dma_start(be12_s[:], be12_d[:])
            ssel_s = csp.tile([128, 5 * NPB], bf16, name="ssel_s")
            nc.sync.dma_start(ssel_s[:], ssel_d[:])
            if not bn_zero:
                bnrow_s = csp.tile([128, L * 2 * 2 * 128], bf16, name="bnrow_s")
                nc.sync.dma_start(bnrow_s[0:1, :], bnrow_d[:])
            if be2_mm:
                be2row_s = csp.tile([128, L * 512], bf16, name="be2row_s")
                nc.sync.dma_start(be2row_s[0:1, :], be2row_d[:])
            ones_s = csp.tile([128, 512], bf16, name="ones_s")
            nc.gpsimd.memset(ones_s[0:1, :], 1.0)
            zcol_s = csp.tile([128, 1], f32, name="zcol_s")
            nc.gpsimd.memset(zcol_s[:], 0.0)

            def we2r_ap(l, kc):
                o = (l * 2 + kc) * 256
                return we2r_s[:, o:o + 256]

            def w1ab_ap(l, sel, kc):
                o = ((l * 2 + sel) * 2 + kc) * 256
                return w1ab_s[:, o:o + 256]

            def wfam_ap(t, l, kc, mc):
                o = ((l * 2 + kc) * 2 + mc) * 128
                return t[:, o:o + 128]

            def bnrow_ap(l, j, mc):
                o = ((l * 2 + j) * 2 + mc) * 128
                return bnrow_s[0:1, o:o + 128]

            def be_ap(j, l, mc):
                o = (j * L + l) * 2 + mc
                return be12_s[:, o:o + 1]

            hT, hbf, n1bf = [], [], []
            for c in range(2):
                hT.append(stp.tile([128, nn], f32, name=f"hT{c}", tag=f"hT{c}"))
                hbf.append(stp.tile([128, nn], bf16, name=f"hbf{c}", tag=f"hbf{c}"))
                n1bf.append(stp.tile([128, nn], bf16, name=f"n1b{c}", tag=f"n1b{c}"))
            aggbf = stp.tile([128, 2 * nn], bf16, name="aggbf", tag="aggbf")

            if not hw_loop:
                for c in range(2):
                    for t in range(nt):
                        sl = slice(t * nts, (t + 1) * nts)
                        nc.sync.dma_start(hT[c][:, sl], h0T_d[c][:, sl])
                        nc.gpsimd.tensor_copy(hbf[c][:, sl], hT[c][:, sl])

            crg_s = []
            for b in range(nb):
                t = crgp.tile([128, EPB], bf16, name=f"crg{b}", tag=f"crg{b}")
                nc.sync.dma_start(t[:], crg_d[b])
                crg_s.append(t)

            comb_tiles = [
                [combp.tile([128, 256], bf16, tag=f"comb{l}_{i}",
                            name=f"comb{l}_{i}") for i in range(min(4, nb))]
                for l in range(L)]

            evict_i = [0]

            def evict(out_ap, ps_ap, relu, bias_ap, dve_of_8):
                use_dve = (evict_i[0] % 8) < dve_of_8
                evict_i[0] += 1
                if relu:
                    if use_dve and bias_ap is None:
                        nc.vector.scalar_tensor_tensor(
                            out_ap, ps_ap, 0.0,
                            zcol_s[:, 0:1].to_broadcast(
                                [out_ap.shape[0], out_ap.free_size()]),
                            op0=ALU.max, op1=ALU.add)
                    elif use_dve:
                        be, nbe = bias_ap
                        nc.vector.scalar_tensor_tensor(
                            out_ap, ps_ap, nbe,
                            be.to_broadcast([out_ap.shape[0], out_ap.free_size()]),
                            op0=ALU.max, op1=ALU.add)
                    else:
                        nc.scalar.activation(out_ap, ps_ap, AF.Relu,
                                             bias=(bias_ap[0] if bias_ap else 0.0))
                else:
                    if use_dve:
                        nc.vector.tensor_copy(out_ap, ps_ap)
                    else:
                        nc.scalar.copy(out_ap, ps_ap)

            # pass-B psum layout inside a [128, 1536] tile (3 banks):
            #   (mc, e0, psum_off, width)
            B_CHUNKS = [(0, 0, 0, 256), (0, 256, 256, 256), (0, 512, 512, 128),
                        (1, 0, 640, 128), (1, 128, 768, 256), (1, 384, 1024, 256)]

            loop_ctx = (tc.For_i(0, reps, 1) if hw_loop else contextlib.nullcontext())
            rep_range = range(1 if hw_loop else reps)
            with loop_ctx:
             for rep in rep_range:
              if hw_loop or rep > 0:
                for c in range(2):
                    for t in range(nt):
                        sl = slice(t * nts, (t + 1) * nts)
                        nc.sync.dma_start(hT[c][:, sl], h0T_d[c][:, sl])
                        nc.gpsimd.tensor_copy(hbf[c][:, sl], hT[c][:, sl])
              for l in range(L):
                for i in range(min(4, nb)):
                    nc.gpsimd.tensor_copy(
                        comb_tiles[l][i][64:128, :],
                        w1dp_s[64:128, l * 256:(l + 1) * 256])
                for g in range(nb // 4):
                    agg_ps = psaggp.tile([128, 256], f32, tag="agg",
                                         name=f"agg{l}_{g}")
                    # ---- pass A: hA/hB for 4 blocks
                    for bi in range(4):
                        b = g * 4 + bi
                        ps_ab = bigp.tile([128, 1536], f32, tag="big",
                                          name=f"ab{l}_{b}")
                        for kc in range(2):
                            for sel, pos0 in ((0, 0), (1, 32)):
                                nc.tensor.matmul(
                                    ps_ab[pos0:pos0 + 32, 0:256],
                                    lhsT=hbf[kc][:, b * NPB:(b + 1) * NPB],
                                    rhs=w1ab_ap(l, sel, kc),
                                    start=(kc == 0), stop=(kc == 1),
                                    tile_position=(0, pos0),
                                    skip_group_check=True)
                        comb = comb_tiles[l][b % 4]
                        evict(comb[0:64, :], ps_ab[0:64, 0:256], False, None,
                              comb_dve_of_8)
                    # ---- pass B + merged m1 eviction
                    m1ts = {}
                    for bi in range(4):
                        b = g * 4 + bi
                        comb = comb_tiles[l][b % 4]
                        psB = bigp.tile([128, 1536], f32, tag="big",
                                        name=f"psB{l}_{b}")
                        for mc, e0, off, w in B_CHUNKS:
                            nc.tensor.matmul(
                                psB[:, off:off + w],
                                lhsT=comb[:, mc * 128:(mc + 1) * 128],
                                rhs=crg_s[b][:, e0:e0 + w],
                                start=True, stop=True)
                        m1 = m1p.tile([128, 2 * EPB], bf16, tag="m1",
                                      name=f"m1_{l}_{b}")
                        m1ts[bi] = m1
                        if be1_zero:
                            evict(m1[:], psB[:, 0:1280], True, None, m1_dve_of_8)
                        else:
                            evict(m1[:, 0:640], psB[:, 0:640], True,
                                  (be_ap(0, l, 0), be_ap(3, l, 0)), m1_dve_of_8)
                            evict(m1[:, 640:1280], psB[:, 640:1280], True,
                                  (be_ap(0, l, 1), be_ap(3, l, 1)), m1_dve_of_8)
                    # ---- pass C + merged m2 eviction
                    m2ss = {}
                    for bi in range(4):
                        b = g * 4 + bi
                        m1 = m1ts[bi]
                        psC = bigp.tile([128, 1536], f32, tag="big",
                                        name=f"psC{l}_{b}")
                        for ec in range(5):
                            if be2_mm:
                                nc.tensor.matmul(
                                    psC[:, ec * 256:(ec + 1) * 256],
                                    lhsT=ones_s[0:1, 0:128],
                                    rhs=be2row_s[0:1, l * 512:l * 512 + 256],
                                    start=True, stop=False)
                            for kc in range(2):
                                nc.tensor.matmul(
                                    psC[:, ec * 256:(ec + 1) * 256],
                                    lhsT=m1[:, kc * EPB + ec * 128:
                                            kc * EPB + (ec + 1) * 128],
                                    rhs=we2r_ap(l, kc),
                                    start=(kc == 0 and not be2_mm),
                                    stop=(kc == 1))
                        m2 = m2p.tile([128, 1280], bf16, tag="m2",
                                      name=f"m2_{l}_{b}")
                        m2ss[bi] = m2
                        evict(m2[:], psC[:, 0:1280], True, None, m2_dve_of_8)
                    # ---- pass D: PE segment-sum into agg psum
                    for bi in range(4):
                        m2 = m2ss[bi]
                        for mc in range(2):
                            for ec in range(5):
                                nc.tensor.matmul(
                                    agg_ps[:, mc * 128 + bi * 32:
                                           mc * 128 + bi * 32 + 32],
                                    lhsT=m2[:, ec * 256 + mc * 128:
                                            ec * 256 + (mc + 1) * 128],
                                    rhs=ssel_s[:, ec * NPB:(ec + 1) * NPB],
                                    start=(ec == 0), stop=(ec == 4))
                    # ---- merged agg eviction -> aggbf [128, (kc, nn)]
                    evict(aggbf[:].rearrange("p (k n) -> p k n", k=2)[
                              :, :, g * 128:(g + 1) * 128],
                          agg_ps[:].rearrange("p (k n) -> p k n", k=2),
                          False, None, agg_dve_of_8)
                # ---------------- node phase
                for mc in range(2):
                    for th in range(nt // 2):
                        psn = bigp.tile([128, 1536], f32, tag="big",
                                        name=f"n1_{l}_{mc}_{th}")
                        for t2 in range(2):
                            t = th * 2 + t2
                            sl = slice(t * nts, (t + 1) * nts)
                            o = t2 * nts
                            first = True
                            if not bn_zero:
                                nc.tensor.matmul(psn[:, o:o + nts],
                                                 lhsT=bnrow_ap(l, 0, mc),
                                                 rhs=ones_s[0:1, 0:nts],
                                                 start=True, stop=False)
                                first = False
                            for kc in range(2):
                                nc.tensor.matmul(psn[:, o:o + nts],
                                                 lhsT=wfam_ap(wn1a_s, l, kc, mc),
                                                 rhs=hbf[kc][:, sl],
                                                 start=first, stop=False)
                                first = False
                                nc.tensor.matmul(
                                    psn[:, o:o + nts],
                                    lhsT=wfam_ap(wn1b_s, l, kc, mc),
                                    rhs=aggbf[:, kc * nn + t * nts:
                                              kc * nn + (t + 1) * nts],
                                    start=False, stop=(kc == 1))
                        sl2 = slice(th * 2 * nts, (th + 1) * 2 * nts)
                        if (evict_i[0] % 8) < n1_dve_of_8:
                            nc.vector.scalar_tensor_tensor(
                                n1bf[mc][:, sl2], psn[:, 0:2 * nts], 0.0,
                                zcol_s[:, 0:1].to_broadcast([128, 2 * nts]),
                                op0=ALU.max, op1=ALU.add)
                        else:
                            nc.scalar.activation(n1bf[mc][:, sl2],
                                                 psn[:, 0:2 * nts], AF.Relu)
                        evict_i[0] += 1
                for mc in range(2):
                    for th in range(nt // 2):
                        pso = bigp.tile([128, 1536], f32, tag="big",
                                        name=f"n2_{l}_{mc}_{th}")
                        for t2 in range(2):
                            t = th * 2 + t2
                            sl = slice(t * nts, (t + 1) * nts)
                            o = t2 * nts
                            first = True
                            if not bn_zero:
                                nc.tensor.matmul(pso[:, o:o + nts],
                                                 lhsT=bnrow_ap(l, 1, mc),
                                                 rhs=ones_s[0:1, 0:nts],
                                                 start=True, stop=False)
                                first = False
                            for kc in range(2):
                                nc.tensor.matmul(pso[:, o:o + nts],
                                                 lhsT=wfam_ap(wn2_s, l, kc, mc),
                                                 rhs=n1bf[kc][:, sl],
                                                 start=first, stop=(kc == 1))
                                first = False
                        sl2 = slice(th * 2 * nts, (th + 1) * 2 * nts)
                        nc.vector.scalar_tensor_tensor(
                            hT[mc][:, sl2], hT[mc][:, sl2], 2.0,
                            pso[:, 0:2 * nts], op0=ALU.mult, op1=ALU.add)
                        if l + 1 < L:
                            nc.gpsimd.tensor_copy(hbf[mc][:, sl2], hT[mc][:, sl2])
              # ---------------- pooling
              for mc in range(2):
                pool_t = stp.tile([128, nb], f32, tag=f"pool{mc}", name=f"pool{mc}")
                nc.vector.tensor_reduce(pool_t[:],
                                        hT[mc][:].rearrange("p (n k) -> p n k", k=NPB),
                                        axis=mybir.AxisListType.X, op=ALU.add)
                nc.scalar.mul(pool_t[:], pool_t[:], 1.0 / NPB)
                nc.sync.dma_start(out_d[mc], pool_t[:])
    return nc


# ===================================================================== entry

_CACHE = {}


def _get_runner(be2_mm, bn_zero, be1_zero):
    key = ("runner", be2_mm, bn_zero, be1_zero)
    if key not in _CACHE:
        apply_tilefix()
        nc = build_nc_v3(nb=BPC, be2_mm=be2_mm, bn_zero=bn_zero,
                         be1_zero=be1_zero,
                         comb_dve_of_8=4, m1_dve_of_8=4, m2_dve_of_8=4,
                         agg_dve_of_8=4, n1_dve_of_8=0)
        split_waits(nc, cap=1, cap_sp=1)
        _CACHE[key] = nc
    return _CACHE[key]


def kernel(**inputs):
    """Full inputs in (as in reference.setup_inputs), full [B, 256] f32 out."""
    np_inputs = {k: np.asarray(v) for k, v in inputs.items()}
    per_core = host_prep(**np_inputs)
    be2_mm = bool(per_core[0]["be2_nonzero"][0])
    bn_zero = not (np.any(np_inputs["bn1"]) or np.any(np_inputs["bn2"]))
    be1_zero = not np.any(np_inputs["be1"])
    nc = _get_runner(be2_mm, bn_zero, be1_zero)

    import concourse.mybir as mybir
    from concourse.bass_utils import run_bass_kernel_spmd
    declared = set()
    for alloc in nc.m.functions[0].allocations:
        if isinstance(alloc, mybir.MemoryLocationSet) and alloc.kind == "ExternalInput":
            declared.add(alloc.memorylocations[0].name)
    in_maps = [{k: v for k, v in m.items() if k in declared} for m in per_core]
    res = run_bass_kernel_spmd(nc, in_maps, core_ids=list(range(NCORES)))
    return host_unshard(res.results).astype(np.float32)

